# revision 8
# baseline (speedup 1.0000x reference)
"""Trainium2 Bass kernel for nn_LocalPoolPointNet (gnn_message_passing).

Sharding strategy (hardcoded):
  - 8 NeuronCores = 4 batches x 2 z-halves of the 64^3 grid. Points are
    sharded to the core owning their voxel's z-half, so every segment_max
    is core-local (a voxel's points all live on exactly one core) and no
    collective is needed.
  - Within a core, 4 "streams" (8 z-slices each) are folded across the 128
    SBUF partitions: partition 32*q + ch holds channel ch of stream q.
    Matmuls use block-diagonal [128,128] stationary weights so all 4
    streams multiply in a single PE pass at full array width.
  - Points are binned by voxel into fixed-size windows (slot class sizes
    1,2,3,4,6,8,... padded by duplicating a point of the same voxel, which
    is max-neutral). segment_max becomes a windowed DVE tensor_reduce(max)
    and the gather-back is a stride-0 access-pattern broadcast consumed
    directly by the PE as the moving matmul operand (no data movement).
    Single-point voxels (the majority) skip pooling entirely: their pooled
    value equals the point value, folded in by pre-summing the net/pooled
    weight blocks.
  - The final dense [ch, 64^3] grid is produced by GPSIMD ap_gather from
    the compressed per-voxel table via a host-built rank map (empty voxels
    hit a zero sentinel), then DMA'd out channel-major.
"""

import os
import sys

sys.path.insert(0, "/opt/trn_rl_repo")

import numpy as np

R = 64
B = 4
N = 100000
NB = 5
NCORES = 8
NSTREAM = 4
VOX_PER_STREAM = 64 * 64 * 8  # 32768
CHUNK = 512
CLASS_SIZES = (1, 2, 3, 4, 6, 8, 12, 16, 24, 32, 48, 64, 96, 128, 160, 192,
               224, 256, 320, 384, 448, 512)
GRID_CHUNK = 2048  # columns per ap_gather / output DMA chunk

# fp32 fits SBUF only for compact layouts; clustered data needs bf16
FP32_MAX_L = 13824


def _pick_bf16(L):
    ov = os.environ.get("KERNEL_MM_BF16")
    if ov is not None:
        return ov == "1"
    return L > FP32_MAX_L


def _coords2index_np(p):
    """Exact float32 replica of reference._coords2index."""
    p = np.asarray(p, dtype=np.float32)
    pn = np.clip(p + np.float32(1.0), np.float32(0.0), np.float32(2.0 - 0.0001))
    xi = ((pn / np.float32(2.0)) * np.float32(R)).astype(np.int32)
    return xi[..., 0] + R * (xi[..., 1] + R * xi[..., 2])


def _class_of(occ):
    for k in CLASS_SIZES:
        if occ <= k:
            return k
    raise ValueError(f"voxel occupancy {occ} too large")


def _bin_streams(p_all):
    """Host-side sharding: bin points by (core, stream, voxel)."""
    idx_all = _coords2index_np(p_all)  # [B, N]
    binned = []
    for b in range(B):
        idx_b = idx_all[b]
        z = idx_b >> 12
        for h in range(2):
            streams = []
            for q in range(NSTREAM):
                z0 = 32 * h + 8 * q
                sel = np.nonzero((z >= z0) & (z < z0 + 8))[0]
                vloc = idx_b[sel] - 4096 * z0
                order = np.argsort(vloc, kind="stable")
                sel, vloc = sel[order], vloc[order]
                uvox, starts, counts = np.unique(
                    vloc, return_index=True, return_counts=True)
                by_class = {}
                for ui in range(len(uvox)):
                    by_class.setdefault(_class_of(counts[ui]), []).append(ui)
                streams.append(dict(sel=sel, uvox=uvox, starts=starts,
                                    counts=counts, by_class=by_class))
            binned.append(streams)
    return binned


def _build_layout(binned):
    """Cross-core/stream padded class layout.

    Returns [(k, nwin, wins_per_chunk)], slot total L, window total V.
    Each class region is a whole number of chunks of wins_per_chunk
    windows (chunk columns = wins_per_chunk * k <= 512)."""
    classes = sorted({k for cs in binned for s in cs for k in s["by_class"]})
    layout = []
    for k in classes:
        wpc = CHUNK // k
        nw = max(len(s["by_class"].get(k, ())) for cs in binned for s in cs)
        nw = -(-nw // wpc) * wpc
        layout.append((k, nw, wpc))
    L = sum(k * nw for k, nw, _ in layout)
    V = sum(nw for _, nw, _ in layout)
    return layout, L, V


def _build_core_inputs(p_all, binned, layout, L, V):
    assert V + 1 <= 32768
    cores = []
    for core in range(NCORES):
        b, h = divmod(core, 2)
        p_f4 = np.zeros((12, L), dtype=np.float32)
        rank_map = np.full((NSTREAM, VOX_PER_STREAM), V, dtype=np.int32)
        for q in range(NSTREAM):
            s = binned[core][q]
            sel, uvox = s["sel"], s["uvox"]
            starts, counts = s["starts"], s["counts"]
            assert len(sel) > 0
            slot_pts = np.full(L, sel[0], dtype=np.int64)
            off_slot = 0
            off_win = 0
            for k, nw, _ in layout:
                for wi, ui in enumerate(s["by_class"].get(k, ())):
                    st, ct = starts[ui], counts[ui]
                    pts = sel[st:st + ct]
                    sl = off_slot + wi * k
                    slot_pts[sl:sl + k] = pts[0]
                    slot_pts[sl:sl + ct] = pts
                    rank_map[q, uvox[ui]] = off_win + wi
                off_slot += k * nw
                off_win += nw
            p_f4[3 * q:3 * q + 3, :] = p_all[b, slot_pts, :].T
        # wrap rank map for ap_gather: chunks of GRID_CHUNK idxs,
        # element e of a chunk -> partition 16*j + e%16, col e//16
        nchunks = VOX_PER_STREAM // GRID_CHUNK
        ccols = GRID_CHUNK // 16
        idx_w = np.zeros((128, nchunks * ccols), dtype=np.int16)
        for q in range(NSTREAM):
            w = rank_map[q].astype(np.int16).reshape(nchunks, ccols, 16)
            flat = np.concatenate([w[c].T for c in range(nchunks)], axis=1)
            for j in (2 * q, 2 * q + 1):
                idx_w[16 * j:16 * j + 16, :] = flat
        cores.append(dict(p_f4=p_f4, idx_w=idx_w))
    return cores


def _bd4(w):
    out = np.zeros((128, 128), dtype=np.float32)
    for q in range(4):
        out[32 * q:32 * q + 32, 32 * q:32 * q + 32] = w
    return out


def _bias_f4(bvec):
    return np.tile(np.asarray(bvec, np.float32), 4).reshape(128, 1)


def _build_weights(inp):
    W = {}
    W_pos = np.asarray(inp["W_pos"], np.float32)
    for half, sl in (("lo", slice(0, 32)), ("hi", slice(32, 64))):
        w = np.zeros((12, 128), dtype=np.float32)
        for q in range(4):
            w[3 * q:3 * q + 3, 32 * q:32 * q + 32] = W_pos[:, sl]
        W[f"wpos_{half}"] = w
    W["bpos_lo"] = _bias_f4(np.asarray(inp["b_pos"], np.float32)[:32])
    W["bpos_hi"] = _bias_f4(np.asarray(inp["b_pos"], np.float32)[32:])
    W0 = np.asarray(inp["W0"], np.float32)
    W1 = np.asarray(inp["W1"], np.float32)
    Ws = np.asarray(inp["Ws"], np.float32)
    for i in range(NB):
        W[f"w0a_{i}"] = _bd4(W0[i, :32])
        W[f"w0b_{i}"] = _bd4(W0[i, 32:])
        W[f"w0ab_{i}"] = _bd4(W0[i, :32] + W0[i, 32:])
        W[f"w1_{i}"] = _bd4(W1[i])
        W[f"wsa_{i}"] = _bd4(Ws[i, :32])
        W[f"wsb_{i}"] = _bd4(Ws[i, 32:])
        W[f"wsab_{i}"] = _bd4(Ws[i, :32] + Ws[i, 32:])
        W[f"b0_{i}"] = _bias_f4(inp["b0"][i])
        W[f"b1_{i}"] = _bias_f4(inp["b1"][i])
    W["wc"] = _bd4(np.asarray(inp["W_c"], np.float32))
    W["bc"] = _bias_f4(inp["b_c"])
    return W


WNAMES = (["wpos_lo", "wpos_hi", "wc"]
          + [f"{nm}_{i}" for i in range(NB)
             for nm in ("w0a", "w0b", "w0ab", "w1", "wsa", "wsb", "wsab")])
BNAMES = (["bpos_lo", "bpos_hi", "bc"]
          + [f"b0_{i}" for i in range(NB)] + [f"b1_{i}" for i in range(NB)])


def _emit_program(layout, L, V, MM_BF16):
    from concourse import bacc, mybir, tile, library_config

    f32 = mybir.dt.float32
    mmdt = mybir.dt.bfloat16 if MM_BF16 else f32
    add = mybir.AluOpType.add
    relu = mybir.ActivationFunctionType.Relu

    nc = bacc.Bacc("TRN2", target_bir_lowering=False, debug=False,
                   num_devices=NCORES)

    d_p = nc.dram_tensor("p_f4", [12, L], mmdt, kind="ExternalInput")
    nidx = (VOX_PER_STREAM // GRID_CHUNK) * (GRID_CHUNK // 16)
    d_idx = nc.dram_tensor("idx_w", [128, nidx], mybir.dt.int16,
                           kind="ExternalInput")
    d_w = {}
    for nm in WNAMES:
        shape = [12, 128] if nm.startswith("wpos") else [128, 128]
        d_w[nm] = nc.dram_tensor(nm, shape, mmdt, kind="ExternalInput")
    for nm in BNAMES:
        d_w[nm] = nc.dram_tensor(nm, [128, 1], f32, kind="ExternalInput")
    d_out = nc.dram_tensor("grid", [128, VOX_PER_STREAM], f32,
                           kind="ExternalOutput")

    # chunk bookkeeping
    chunk_info = []
    class_off = {}
    off_slot = 0
    off_win = 0
    for k, nw, wpc in layout:
        class_off[k] = (off_slot, off_win, nw)
        for c in range(nw // wpc):
            chunk_info.append(dict(
                k=k, slot0=off_slot + c * wpc * k, win0=off_win + c * wpc,
                nwin=wpc, cols=wpc * k))
        off_slot += nw * k
        off_win += nw

    with tile.TileContext(nc) as tc:
        with tc.tile_pool(name="persist", bufs=1) as pers, \
             tc.tile_pool(name="chunks", bufs=4) as chp, \
             tc.tile_pool(name="mpool", bufs=1) as mp, \
             tc.tile_pool(name="gridp", bufs=2) as grp, \
             tc.tile_pool(name="psum", bufs=2, space="PSUM") as psp:

            # ucode library for the final gather: issue first, its ~110us
            # reload drain overlaps the whole MLP phase
            nc.gpsimd.load_library(library_config.ap_gather)

            sb_w = {}
            for nm in WNAMES:
                shape = [12, 128] if nm.startswith("wpos") else [128, 128]
                t = pers.tile(shape, mmdt, tag=nm)
                nc.sync.dma_start(out=t[:], in_=d_w[nm][:])
                sb_w[nm] = t
            for nm in BNAMES:
                t = pers.tile([128, 1], f32, tag=nm)
                nc.sync.dma_start(out=t[:], in_=d_w[nm][:])
                sb_w[nm] = t
            idx_sb = pers.tile([128, nidx], mybir.dt.int16, tag="idx_sb")
            nc.sync.dma_start(out=idx_sb[:], in_=d_idx[:])

            bigA = pers.tile([128, L], mmdt, tag="bigA")
            bigB = pers.tile([128, L], mmdt, tag="bigB")
            if MM_BF16:
                table_t = pers.tile([128, V + 1], f32, tag="table")
                table = table_t[:]
            else:
                # final phase: cur=bigA (net5), bigB (net4) is dead -> reuse
                table = bigB[:, 0:V + 1]

            def mm(ps, wname, rhs, start, stop):
                nc.tensor.matmul(out=ps, lhsT=sb_w[wname][:], rhs=rhs,
                                 start=start, stop=stop)

            # ---- fused pos MLP + block 0 (chunk-local X) ----
            for ci, info in enumerate(chunk_info):
                s0, cols = info["slot0"], info["cols"]
                p_ch = chp.tile([12, CHUNK], mmdt, tag="p_ch")
                nc.sync.dma_start(out=p_ch[:, :cols],
                                  in_=d_p[:, s0:s0 + cols])
                xlo = chp.tile([128, CHUNK], mmdt, tag="c0")
                xhi = chp.tile([128, CHUNK], mmdt, tag="c1")
                for wn, bn, X in (("wpos_lo", "bpos_lo", xlo),
                                  ("wpos_hi", "bpos_hi", xhi)):
                    ps = psp.tile([128, CHUNK], f32, space="PSUM", tag="psP")
                    mm(ps[:, :cols], wn, p_ch[:, :cols], True, True)
                    nc.vector.tensor_scalar_add(
                        out=X[:, :cols], in0=ps[:, :cols],
                        scalar1=sb_w[bn][:, 0:1])
                rlo = chp.tile([128, CHUNK], mmdt, tag="c2")
                rhi = chp.tile([128, CHUNK], mmdt, tag="c3")
                nc.gpsimd.tensor_scalar_max(out=rlo[:, :cols],
                                            in0=xlo[:, :cols], scalar1=0.0)
                nc.gpsimd.tensor_scalar_max(out=rhi[:, :cols],
                                            in0=xhi[:, :cols], scalar1=0.0)
                psA = psp.tile([128, CHUNK], f32, space="PSUM", tag="psA")
                mm(psA[:, :cols], "w0a_0", rlo[:, :cols], True, False)
                mm(psA[:, :cols], "w0b_0", rhi[:, :cols], False, True)
                ra = chp.tile([128, CHUNK], mmdt, tag="c4")
                nc.scalar.activation(ra[:, :cols], psA[:, :cols], relu,
                                     bias=sb_w["b0_0"][:, 0:1])
                psD = psp.tile([128, CHUNK], f32, space="PSUM", tag="psD")
                mm(psD[:, :cols], "w1_0", ra[:, :cols], True, False)
                mm(psD[:, :cols], "wsa_0", xlo[:, :cols], False, False)
                mm(psD[:, :cols], "wsb_0", xhi[:, :cols], False, True)
                nc.vector.tensor_scalar_add(
                    out=bigA[:, s0:s0 + cols], in0=psD[:, :cols],
                    scalar1=sb_w["b1_0"][:, 0:1])

            # ---- blocks 1..4 with pooling ----
            cur, nxt = bigA, bigB
            for i in range(1, NB):
                M, RM = {}, {}
                for k, nw, wpc in layout:
                    if k == 1:
                        continue
                    slot0, win0, nwk = class_off[k]
                    m = mp.tile([128, nwk], mmdt, tag=f"M_{k}")
                    nc.vector.tensor_reduce(
                        out=m[:],
                        in_=cur[:, slot0:slot0 + nwk * k]
                        .rearrange("p (w k) -> p w k", k=k),
                        axis=mybir.AxisListType.X, op=mybir.AluOpType.max)
                    rm = mp.tile([128, nwk], mmdt, tag=f"RM_{k}")
                    nc.scalar.activation(rm[:], m[:], relu)
                    M[k], RM[k] = m, rm
                for ci, info in enumerate(chunk_info):
                    k, s0, cols = info["k"], info["slot0"], info["cols"]
                    rn = chp.tile([128, CHUNK], mmdt, tag="c0")
                    nc.gpsimd.tensor_scalar_max(out=rn[:, :cols],
                                                in0=cur[:, s0:s0 + cols],
                                                scalar1=0.0)
                    psA = psp.tile([128, CHUNK], f32, space="PSUM", tag="psA")
                    psD = psp.tile([128, CHUNK], f32, space="PSUM", tag="psD")
                    if k == 1:
                        mm(psA[:, :cols], f"w0ab_{i}", rn[:, :cols],
                           True, True)
                    else:
                        wrel = info["win0"] - class_off[k][1]
                        mm(psA[:, :cols], f"w0a_{i}", rn[:, :cols],
                           True, False)
                        bc = RM[k][:, wrel:wrel + info["nwin"]] \
                            .unsqueeze(2).to_broadcast([128, info["nwin"], k])
                        mm(psA[:, :cols], f"w0b_{i}", bc, False, True)
                    ra = chp.tile([128, CHUNK], mmdt, tag="c4")
                    nc.scalar.activation(ra[:, :cols], psA[:, :cols], relu,
                                         bias=sb_w[f"b0_{i}"][:, 0:1])
                    mm(psD[:, :cols], f"w1_{i}", ra[:, :cols], True, False)
                    if k == 1:
                        mm(psD[:, :cols], f"wsab_{i}", cur[:, s0:s0 + cols],
                           False, True)
                    else:
                        wrel = info["win0"] - class_off[k][1]
                        mm(psD[:, :cols], f"wsa_{i}", cur[:, s0:s0 + cols],
                           False, False)
                        bc = M[k][:, wrel:wrel + info["nwin"]] \
                            .unsqueeze(2).to_broadcast([128, info["nwin"], k])
                        mm(psD[:, :cols], f"wsb_{i}", bc, False, True)
                    nc.vector.tensor_scalar_add(
                        out=nxt[:, s0:s0 + cols], in0=psD[:, :cols],
                        scalar1=sb_w[f"b1_{i}"][:, 0:1])
                cur, nxt = nxt, cur

            # ---- final: c = net@W_c + b_c, per-voxel max, relu -> table ----
            for ci, info in enumerate(chunk_info):
                k, s0, cols = info["k"], info["slot0"], info["cols"]
                w0, nwin = info["win0"], info["nwin"]
                ps = psp.tile([128, CHUNK], f32, space="PSUM", tag="psC")
                mm(ps[:, :cols], "wc", cur[:, s0:s0 + cols], True, True)
                if k == 1:
                    nc.scalar.activation(table[:, w0:w0 + nwin],
                                         ps[:, :cols], relu,
                                         bias=sb_w["bc"][:, 0:1])
                else:
                    mc = chp.tile([128, CHUNK], f32, tag="c0")
                    nc.vector.tensor_reduce(
                        out=mc[:, :nwin],
                        in_=ps[:, :cols].rearrange("p (w k) -> p w k", k=k),
                        axis=mybir.AxisListType.X, op=mybir.AluOpType.max)
                    nc.scalar.activation(table[:, w0:w0 + nwin],
                                         mc[:, :nwin], relu,
                                         bias=sb_w["bc"][:, 0:1])
            nc.vector.memset(table[:, V:V + 1], 0.0)

            # ---- dense grid via ap_gather, stream out ----
            ccols = GRID_CHUNK // 16
            for zc in range(VOX_PER_STREAM // GRID_CHUNK):
                g = grp.tile([128, GRID_CHUNK], f32, tag="grid_chunk")
                nc.gpsimd.ap_gather(
                    g[:], table[:], idx_sb[:, ccols * zc:ccols * (zc + 1)],
                    channels=128, num_elems=V + 1, d=1, num_idxs=GRID_CHUNK)
                nc.sync.dma_start(
                    out=d_out[:, GRID_CHUNK * zc:GRID_CHUNK * (zc + 1)],
                    in_=g[:])

    nc.compile()
    return nc


_CACHE = {}


def _to_mm_dtype(arr, MM_BF16):
    if MM_BF16:
        import ml_dtypes
        return np.asarray(arr).astype(ml_dtypes.bfloat16)
    return np.asarray(arr, np.float32)


def kernel(**inputs):
    from concourse.bass_utils import run_bass_kernel_spmd

    p_all = np.asarray(inputs["p"], np.float32)
    binned = _bin_streams(p_all)
    layout, L, V = _build_layout(binned)
    cores = _build_core_inputs(p_all, binned, layout, L, V)
    W = _build_weights(inputs)

    MM_BF16 = _pick_bf16(L)
    key = (tuple(layout), L, V, MM_BF16)
    if key not in _CACHE:
        _CACHE[key] = _emit_program(layout, L, V, MM_BF16)
    nc = _CACHE[key]

    in_maps = []
    for core in range(NCORES):
        m = {"p_f4": _to_mm_dtype(cores[core]["p_f4"], MM_BF16),
             "idx_w": cores[core]["idx_w"]}
        for nm in WNAMES:
            m[nm] = _to_mm_dtype(W[nm], MM_BF16)
        for nm in BNAMES:
            m[nm] = W[nm].astype(np.float32)
        in_maps.append(m)

    res = run_bass_kernel_spmd(nc, in_maps, list(range(NCORES)))

    out = np.zeros((B, 32, R, R, R), dtype=np.float32)
    for core in range(NCORES):
        b, h = divmod(core, 2)
        g = res.results[core]["grid"]  # [128, 32768]
        g = g.reshape(4, 32, 8, 64, 64).transpose(1, 0, 2, 3, 4)
        out[b, :, 32 * h:32 * h + 32] = g.reshape(32, 32, 64, 64)
    return out


# revision 9
# speedup vs baseline: 2.0704x; 2.0704x over previous
"""Trainium2 Bass kernel for nn_LocalPoolPointNet (gnn_message_passing).

Sharding strategy (hardcoded):
  - 8 NeuronCores = 4 batches x 2 z-halves of the 64^3 grid. Points are
    sharded to the core owning their voxel's z-half, so every segment_max
    is core-local (a voxel's points all live on exactly one core) and no
    collective is needed.
  - Within a core, 4 "streams" (8 z-slices each) are folded across the 128
    SBUF partitions: partition 32*q + ch holds channel ch of stream q.
    Matmuls use block-diagonal [128,128] stationary weights so all 4
    streams multiply in a single PE pass at full array width.
  - Points are binned by voxel into fixed-size windows (slot class sizes
    1,2,3,4,6,8,... padded by duplicating a point of the same voxel, which
    is max-neutral). segment_max becomes a windowed DVE tensor_reduce(max)
    and the gather-back is a stride-0 access-pattern broadcast consumed
    directly by the PE as the moving matmul operand (no data movement).
    Single-point voxels (the majority) skip pooling entirely: their pooled
    value equals the point value, folded in by pre-summing the net/pooled
    weight blocks.
  - The final dense [ch, 64^3] grid is produced by GPSIMD ap_gather from
    the compressed per-voxel table via a host-built rank map (empty voxels
    hit a zero sentinel), then DMA'd out channel-major.
"""

import os
import sys

sys.path.insert(0, "/opt/trn_rl_repo")

import numpy as np

R = 64
B = 4
N = 100000
NB = 5
NCORES = 8
NSTREAM = 4
VOX_PER_STREAM = 64 * 64 * 8  # 32768
CHUNK = 512
CLASS_SIZES = (1, 2, 3, 4, 6, 8, 12, 16, 24, 32, 48, 64, 96, 128, 160, 192,
               224, 256, 320, 384, 448, 512)
GRID_CHUNK = 2048  # columns per ap_gather / output DMA chunk

# fp32 fits SBUF only for compact layouts; clustered data needs bf16
FP32_MAX_L = 13824


def _pick_bf16(L):
    ov = os.environ.get("KERNEL_MM_BF16")
    if ov is not None:
        return ov == "1"
    return L > FP32_MAX_L


def _coords2index_np(p):
    """Exact float32 replica of reference._coords2index."""
    p = np.asarray(p, dtype=np.float32)
    pn = np.clip(p + np.float32(1.0), np.float32(0.0), np.float32(2.0 - 0.0001))
    xi = ((pn / np.float32(2.0)) * np.float32(R)).astype(np.int32)
    return xi[..., 0] + R * (xi[..., 1] + R * xi[..., 2])


def _class_of(occ):
    for k in CLASS_SIZES:
        if occ <= k:
            return k
    raise ValueError(f"voxel occupancy {occ} too large")


def _bin_streams(p_all):
    """Host-side sharding: bin points by (core, stream, voxel)."""
    idx_all = _coords2index_np(p_all)  # [B, N]
    binned = []
    for b in range(B):
        idx_b = idx_all[b]
        z = idx_b >> 12
        for h in range(2):
            streams = []
            for q in range(NSTREAM):
                z0 = 32 * h + 8 * q
                sel = np.nonzero((z >= z0) & (z < z0 + 8))[0]
                vloc = idx_b[sel] - 4096 * z0
                order = np.argsort(vloc, kind="stable")
                sel, vloc = sel[order], vloc[order]
                uvox, starts, counts = np.unique(
                    vloc, return_index=True, return_counts=True)
                by_class = {}
                for ui in range(len(uvox)):
                    by_class.setdefault(_class_of(counts[ui]), []).append(ui)
                streams.append(dict(sel=sel, uvox=uvox, starts=starts,
                                    counts=counts, by_class=by_class))
            binned.append(streams)
    return binned


def _build_layout(binned):
    """Cross-core/stream padded class layout.

    Returns [(k, nwin, wins_per_chunk)], slot total L, window total V.
    Each class region is a whole number of chunks of wins_per_chunk
    windows (chunk columns = wins_per_chunk * k <= 512)."""
    classes = sorted({k for cs in binned for s in cs for k in s["by_class"]})
    layout = []
    for k in classes:
        wpc = CHUNK // k
        nw = max(len(s["by_class"].get(k, ())) for cs in binned for s in cs)
        nw = -(-nw // wpc) * wpc
        layout.append((k, nw, wpc))
    L = sum(k * nw for k, nw, _ in layout)
    V = sum(nw for _, nw, _ in layout)
    return layout, L, V


def _build_core_inputs(p_all, binned, layout, L, V):
    assert V + 1 <= 32768
    cores = []
    for core in range(NCORES):
        b, h = divmod(core, 2)
        p_f4 = np.zeros((12, L), dtype=np.float32)
        rank_map = np.full((NSTREAM, VOX_PER_STREAM), V, dtype=np.int32)
        for q in range(NSTREAM):
            s = binned[core][q]
            sel, uvox = s["sel"], s["uvox"]
            starts, counts = s["starts"], s["counts"]
            assert len(sel) > 0
            slot_pts = np.full(L, sel[0], dtype=np.int64)
            off_slot = 0
            off_win = 0
            for k, nw, _ in layout:
                for wi, ui in enumerate(s["by_class"].get(k, ())):
                    st, ct = starts[ui], counts[ui]
                    pts = sel[st:st + ct]
                    sl = off_slot + wi * k
                    slot_pts[sl:sl + k] = pts[0]
                    slot_pts[sl:sl + ct] = pts
                    rank_map[q, uvox[ui]] = off_win + wi
                off_slot += k * nw
                off_win += nw
            p_f4[3 * q:3 * q + 3, :] = p_all[b, slot_pts, :].T
        # wrap rank map for ap_gather: chunks of GRID_CHUNK idxs,
        # element e of a chunk -> partition 16*j + e%16, col e//16
        nchunks = VOX_PER_STREAM // GRID_CHUNK
        ccols = GRID_CHUNK // 16
        idx_w = np.zeros((128, nchunks * ccols), dtype=np.int16)
        for q in range(NSTREAM):
            w = rank_map[q].astype(np.int16).reshape(nchunks, ccols, 16)
            flat = np.concatenate([w[c].T for c in range(nchunks)], axis=1)
            for j in (2 * q, 2 * q + 1):
                idx_w[16 * j:16 * j + 16, :] = flat
        cores.append(dict(p_f4=p_f4, idx_w=idx_w))
    return cores


def _bd4(w):
    out = np.zeros((128, 128), dtype=np.float32)
    for q in range(4):
        out[32 * q:32 * q + 32, 32 * q:32 * q + 32] = w
    return out


def _bias_f4(bvec):
    return np.tile(np.asarray(bvec, np.float32), 4).reshape(128, 1)


def _build_weights(inp):
    W = {}
    W_pos = np.asarray(inp["W_pos"], np.float32)
    for half, sl in (("lo", slice(0, 32)), ("hi", slice(32, 64))):
        w = np.zeros((12, 128), dtype=np.float32)
        for q in range(4):
            w[3 * q:3 * q + 3, 32 * q:32 * q + 32] = W_pos[:, sl]
        W[f"wpos_{half}"] = w
    W["bpos_lo"] = _bias_f4(np.asarray(inp["b_pos"], np.float32)[:32])
    W["bpos_hi"] = _bias_f4(np.asarray(inp["b_pos"], np.float32)[32:])
    W0 = np.asarray(inp["W0"], np.float32)
    W1 = np.asarray(inp["W1"], np.float32)
    Ws = np.asarray(inp["Ws"], np.float32)
    for i in range(NB):
        W[f"w0a_{i}"] = _bd4(W0[i, :32])
        W[f"w0b_{i}"] = _bd4(W0[i, 32:])
        W[f"w0ab_{i}"] = _bd4(W0[i, :32] + W0[i, 32:])
        W[f"w1_{i}"] = _bd4(W1[i])
        W[f"wsa_{i}"] = _bd4(Ws[i, :32])
        W[f"wsb_{i}"] = _bd4(Ws[i, 32:])
        W[f"wsab_{i}"] = _bd4(Ws[i, :32] + Ws[i, 32:])
        W[f"b0_{i}"] = _bias_f4(inp["b0"][i])
        W[f"b1_{i}"] = _bias_f4(inp["b1"][i])
    W["wc"] = _bd4(np.asarray(inp["W_c"], np.float32))
    W["bc"] = _bias_f4(inp["b_c"])
    return W


WNAMES = (["wpos_lo", "wpos_hi", "wc"]
          + [f"{nm}_{i}" for i in range(NB)
             for nm in ("w0a", "w0b", "w0ab", "w1", "wsa", "wsb", "wsab")])
BNAMES = (["bpos_lo", "bpos_hi", "bc"]
          + [f"b0_{i}" for i in range(NB)] + [f"b1_{i}" for i in range(NB)])


def _emit_program(layout, L, V, MM_BF16):
    from concourse import bacc, mybir, tile, library_config

    f32 = mybir.dt.float32
    mmdt = mybir.dt.bfloat16 if MM_BF16 else f32
    add = mybir.AluOpType.add
    relu = mybir.ActivationFunctionType.Relu

    nc = bacc.Bacc("TRN2", target_bir_lowering=False, debug=False,
                   num_devices=NCORES)

    d_p = nc.dram_tensor("p_f4", [12, L], mmdt, kind="ExternalInput")
    nidx = (VOX_PER_STREAM // GRID_CHUNK) * (GRID_CHUNK // 16)
    d_idx = nc.dram_tensor("idx_w", [128, nidx], mybir.dt.int16,
                           kind="ExternalInput")
    d_w = {}
    for nm in WNAMES:
        shape = [12, 128] if nm.startswith("wpos") else [128, 128]
        d_w[nm] = nc.dram_tensor(nm, shape, mmdt, kind="ExternalInput")
    for nm in BNAMES:
        d_w[nm] = nc.dram_tensor(nm, [128, 1], f32, kind="ExternalInput")
    d_out = nc.dram_tensor("grid", [128, VOX_PER_STREAM], f32,
                           kind="ExternalOutput")

    # chunk bookkeeping
    chunk_info = []
    class_off = {}
    off_slot = 0
    off_win = 0
    for k, nw, wpc in layout:
        class_off[k] = (off_slot, off_win, nw)
        for c in range(nw // wpc):
            chunk_info.append(dict(
                k=k, slot0=off_slot + c * wpc * k, win0=off_win + c * wpc,
                nwin=wpc, cols=wpc * k))
        off_slot += nw * k
        off_win += nw

    with tile.TileContext(nc) as tc:
        with tc.tile_pool(name="persist", bufs=1) as pers, \
             tc.tile_pool(name="chunks", bufs=4) as chp, \
             tc.tile_pool(name="mpool", bufs=1) as mp, \
             tc.tile_pool(name="gridp", bufs=2) as grp, \
             tc.tile_pool(name="psum", bufs=2, space="PSUM") as psp:

            # ucode library for the final gather: issue first, its ~110us
            # reload drain overlaps the whole MLP phase
            nc.gpsimd.load_library(library_config.ap_gather)

            sb_w = {}
            for nm in WNAMES:
                shape = [12, 128] if nm.startswith("wpos") else [128, 128]
                t = pers.tile(shape, mmdt, tag=nm)
                nc.sync.dma_start(out=t[:], in_=d_w[nm][:])
                sb_w[nm] = t
            for nm in BNAMES:
                t = pers.tile([128, 1], f32, tag=nm)
                nc.sync.dma_start(out=t[:], in_=d_w[nm][:])
                sb_w[nm] = t
            idx_sb = pers.tile([128, nidx], mybir.dt.int16, tag="idx_sb")
            nc.sync.dma_start(out=idx_sb[:], in_=d_idx[:])

            bigA = pers.tile([128, L], mmdt, tag="bigA")
            bigB = pers.tile([128, L], mmdt, tag="bigB")
            if MM_BF16:
                table_t = pers.tile([128, V + 1], f32, tag="table")
                table = table_t[:]
            else:
                # final phase: cur=bigA (net5), bigB (net4) is dead -> reuse
                table = bigB[:, 0:V + 1]

            def mm(ps, wname, rhs, start, stop):
                nc.tensor.matmul(out=ps, lhsT=sb_w[wname][:], rhs=rhs,
                                 start=start, stop=stop)

            # ---- fused pos MLP + block 0 (chunk-local X) ----
            for ci, info in enumerate(chunk_info):
                s0, cols = info["slot0"], info["cols"]
                p_ch = chp.tile([12, CHUNK], mmdt, tag="p_ch")
                nc.sync.dma_start(out=p_ch[:, :cols],
                                  in_=d_p[:, s0:s0 + cols])
                xlo = chp.tile([128, CHUNK], mmdt, tag="c0")
                xhi = chp.tile([128, CHUNK], mmdt, tag="c1")
                for wn, bn, X in (("wpos_lo", "bpos_lo", xlo),
                                  ("wpos_hi", "bpos_hi", xhi)):
                    ps = psp.tile([128, CHUNK], f32, space="PSUM", tag="psP")
                    mm(ps[:, :cols], wn, p_ch[:, :cols], True, True)
                    nc.vector.tensor_scalar_add(
                        out=X[:, :cols], in0=ps[:, :cols],
                        scalar1=sb_w[bn][:, 0:1])
                rlo = chp.tile([128, CHUNK], mmdt, tag="c2")
                rhi = chp.tile([128, CHUNK], mmdt, tag="c3")
                nc.scalar.activation(rlo[:, :cols], xlo[:, :cols], relu)
                nc.scalar.activation(rhi[:, :cols], xhi[:, :cols], relu)
                psA = psp.tile([128, CHUNK], f32, space="PSUM", tag="psA")
                mm(psA[:, :cols], "w0a_0", rlo[:, :cols], True, False)
                mm(psA[:, :cols], "w0b_0", rhi[:, :cols], False, True)
                ra = chp.tile([128, CHUNK], mmdt, tag="c4")
                nc.scalar.activation(ra[:, :cols], psA[:, :cols], relu,
                                     bias=sb_w["b0_0"][:, 0:1])
                psD = psp.tile([128, CHUNK], f32, space="PSUM", tag="psD")
                mm(psD[:, :cols], "w1_0", ra[:, :cols], True, False)
                mm(psD[:, :cols], "wsa_0", xlo[:, :cols], False, False)
                mm(psD[:, :cols], "wsb_0", xhi[:, :cols], False, True)
                nc.vector.tensor_scalar_add(
                    out=bigA[:, s0:s0 + cols], in0=psD[:, :cols],
                    scalar1=sb_w["b1_0"][:, 0:1])

            # ---- blocks 1..4 with pooling ----
            cur, nxt = bigA, bigB
            for i in range(1, NB):
                M, RM = {}, {}
                for k, nw, wpc in layout:
                    if k == 1:
                        continue
                    slot0, win0, nwk = class_off[k]
                    m = mp.tile([128, nwk], mmdt, tag=f"M_{k}")
                    nc.vector.tensor_reduce(
                        out=m[:],
                        in_=cur[:, slot0:slot0 + nwk * k]
                        .rearrange("p (w k) -> p w k", k=k),
                        axis=mybir.AxisListType.X, op=mybir.AluOpType.max)
                    rm = mp.tile([128, nwk], mmdt, tag=f"RM_{k}")
                    nc.scalar.activation(rm[:], m[:], relu)
                    M[k], RM[k] = m, rm
                for ci, info in enumerate(chunk_info):
                    k, s0, cols = info["k"], info["slot0"], info["cols"]
                    rn = chp.tile([128, CHUNK], mmdt, tag="c0")
                    nc.scalar.activation(rn[:, :cols], cur[:, s0:s0 + cols],
                                         relu)
                    psA = psp.tile([128, CHUNK], f32, space="PSUM", tag="psA")
                    psD = psp.tile([128, CHUNK], f32, space="PSUM", tag="psD")
                    if k == 1:
                        mm(psA[:, :cols], f"w0ab_{i}", rn[:, :cols],
                           True, True)
                    else:
                        wrel = info["win0"] - class_off[k][1]
                        mm(psA[:, :cols], f"w0a_{i}", rn[:, :cols],
                           True, False)
                        bc = RM[k][:, wrel:wrel + info["nwin"]] \
                            .unsqueeze(2).to_broadcast([128, info["nwin"], k])
                        mm(psA[:, :cols], f"w0b_{i}", bc, False, True)
                    ra = chp.tile([128, CHUNK], mmdt, tag="c4")
                    nc.scalar.activation(ra[:, :cols], psA[:, :cols], relu,
                                         bias=sb_w[f"b0_{i}"][:, 0:1])
                    mm(psD[:, :cols], f"w1_{i}", ra[:, :cols], True, False)
                    if k == 1:
                        mm(psD[:, :cols], f"wsab_{i}", cur[:, s0:s0 + cols],
                           False, True)
                    else:
                        wrel = info["win0"] - class_off[k][1]
                        mm(psD[:, :cols], f"wsa_{i}", cur[:, s0:s0 + cols],
                           False, False)
                        bc = M[k][:, wrel:wrel + info["nwin"]] \
                            .unsqueeze(2).to_broadcast([128, info["nwin"], k])
                        mm(psD[:, :cols], f"wsb_{i}", bc, False, True)
                    nc.vector.tensor_scalar_add(
                        out=nxt[:, s0:s0 + cols], in0=psD[:, :cols],
                        scalar1=sb_w[f"b1_{i}"][:, 0:1])
                cur, nxt = nxt, cur

            # ---- final: c = net@W_c + b_c, per-voxel max, relu -> table ----
            for ci, info in enumerate(chunk_info):
                k, s0, cols = info["k"], info["slot0"], info["cols"]
                w0, nwin = info["win0"], info["nwin"]
                ps = psp.tile([128, CHUNK], f32, space="PSUM", tag="psC")
                mm(ps[:, :cols], "wc", cur[:, s0:s0 + cols], True, True)
                if k == 1:
                    nc.scalar.activation(table[:, w0:w0 + nwin],
                                         ps[:, :cols], relu,
                                         bias=sb_w["bc"][:, 0:1])
                else:
                    mc = chp.tile([128, CHUNK], f32, tag="c0")
                    nc.vector.tensor_reduce(
                        out=mc[:, :nwin],
                        in_=ps[:, :cols].rearrange("p (w k) -> p w k", k=k),
                        axis=mybir.AxisListType.X, op=mybir.AluOpType.max)
                    nc.scalar.activation(table[:, w0:w0 + nwin],
                                         mc[:, :nwin], relu,
                                         bias=sb_w["bc"][:, 0:1])
            nc.vector.memset(table[:, V:V + 1], 0.0)

            # ---- dense grid via ap_gather, stream out ----
            ccols = GRID_CHUNK // 16
            for zc in range(VOX_PER_STREAM // GRID_CHUNK):
                g = grp.tile([128, GRID_CHUNK], f32, tag="grid_chunk")
                nc.gpsimd.ap_gather(
                    g[:], table[:], idx_sb[:, ccols * zc:ccols * (zc + 1)],
                    channels=128, num_elems=V + 1, d=1, num_idxs=GRID_CHUNK)
                nc.sync.dma_start(
                    out=d_out[:, GRID_CHUNK * zc:GRID_CHUNK * (zc + 1)],
                    in_=g[:])

    nc.compile()
    return nc


_CACHE = {}


def _to_mm_dtype(arr, MM_BF16):
    if MM_BF16:
        import ml_dtypes
        return np.asarray(arr).astype(ml_dtypes.bfloat16)
    return np.asarray(arr, np.float32)


def kernel(**inputs):
    from concourse.bass_utils import run_bass_kernel_spmd

    p_all = np.asarray(inputs["p"], np.float32)
    binned = _bin_streams(p_all)
    layout, L, V = _build_layout(binned)
    cores = _build_core_inputs(p_all, binned, layout, L, V)
    W = _build_weights(inputs)

    MM_BF16 = _pick_bf16(L)
    key = (tuple(layout), L, V, MM_BF16)
    if key not in _CACHE:
        _CACHE[key] = _emit_program(layout, L, V, MM_BF16)
    nc = _CACHE[key]

    in_maps = []
    for core in range(NCORES):
        m = {"p_f4": _to_mm_dtype(cores[core]["p_f4"], MM_BF16),
             "idx_w": cores[core]["idx_w"]}
        for nm in WNAMES:
            m[nm] = _to_mm_dtype(W[nm], MM_BF16)
        for nm in BNAMES:
            m[nm] = W[nm].astype(np.float32)
        in_maps.append(m)

    res = run_bass_kernel_spmd(nc, in_maps, list(range(NCORES)))

    out = np.zeros((B, 32, R, R, R), dtype=np.float32)
    for core in range(NCORES):
        b, h = divmod(core, 2)
        g = res.results[core]["grid"]  # [128, 32768]
        g = g.reshape(4, 32, 8, 64, 64).transpose(1, 0, 2, 3, 4)
        out[b, :, 32 * h:32 * h + 32] = g.reshape(32, 32, 64, 64)
    return out


# revision 14
# speedup vs baseline: 4.3763x; 2.1137x over previous
"""Trainium2 Bass kernel for nn_LocalPoolPointNet (gnn_message_passing).

Sharding strategy (hardcoded):
  - 8 NeuronCores = 4 batches x 2 z-halves of the 64^3 grid. Points are
    sharded to the core owning their voxel's z-half, so every segment_max
    is core-local (a voxel's points all live on exactly one core) and no
    collective is needed.
  - Within a core, 4 "streams" (8 z-slices each) are folded across the 128
    SBUF partitions: partition 32*q + ch holds channel ch of stream q.
    Matmuls use block-diagonal [128,128] stationary weights so all 4
    streams multiply in a single PE pass at full array width.
  - Points are binned by voxel into fixed-size windows (slot class sizes
    1,2,3,4,6,8,... padded by duplicating a point of the same voxel, which
    is max-neutral). segment_max becomes a windowed DVE tensor_reduce(max)
    and the gather-back is a stride-0 access-pattern broadcast consumed
    directly by the PE as the moving matmul operand (no data movement).
    Single-point voxels (the majority) skip pooling entirely: their pooled
    value equals the point value, folded in by pre-summing the net/pooled
    weight blocks.
  - The final dense [64^3, ch] grid is zero-filled by early (overlapped)
    DMAs; the compressed per-voxel table is PE-transposed to voxel-major
    and scattered sparsely with indirect DMA (pad windows skipped via
    bounds_check).
"""

import os
import sys

sys.path.insert(0, "/opt/trn_rl_repo")

import numpy as np

R = 64
B = 4
N = 100000
NB = 5
NCORES = 8
NSTREAM = 4
VOX_PER_STREAM = 64 * 64 * 8  # 32768
CHUNK = 512
CLASS_SIZES = (1, 2, 3, 4, 6, 8, 12, 16, 24, 32, 48, 64, 96, 128, 160, 192,
               224, 256, 320, 384, 448, 512)
GRID_CHUNK = 2048  # columns per ap_gather / output DMA chunk

# fp32 fits SBUF only for compact layouts; clustered data needs bf16
FP32_MAX_L = 13824


def _pick_bf16(L):
    ov = os.environ.get("KERNEL_MM_BF16")
    if ov is not None:
        return ov == "1"
    return L > FP32_MAX_L


def _coords2index_np(p):
    """Exact float32 replica of reference._coords2index."""
    p = np.asarray(p, dtype=np.float32)
    pn = np.clip(p + np.float32(1.0), np.float32(0.0), np.float32(2.0 - 0.0001))
    xi = ((pn / np.float32(2.0)) * np.float32(R)).astype(np.int32)
    return xi[..., 0] + R * (xi[..., 1] + R * xi[..., 2])


def _class_of(occ):
    for k in CLASS_SIZES:
        if occ <= k:
            return k
    raise ValueError(f"voxel occupancy {occ} too large")


def _bin_streams(p_all):
    """Host-side sharding: bin points by (core, stream, voxel)."""
    idx_all = _coords2index_np(p_all)  # [B, N]
    binned = []
    for b in range(B):
        idx_b = idx_all[b]
        z = idx_b >> 12
        for h in range(2):
            streams = []
            for q in range(NSTREAM):
                z0 = 32 * h + 8 * q
                sel = np.nonzero((z >= z0) & (z < z0 + 8))[0]
                vloc = idx_b[sel] - 4096 * z0
                order = np.argsort(vloc, kind="stable")
                sel, vloc = sel[order], vloc[order]
                uvox, starts, counts = np.unique(
                    vloc, return_index=True, return_counts=True)
                by_class = {}
                for ui in range(len(uvox)):
                    by_class.setdefault(_class_of(counts[ui]), []).append(ui)
                streams.append(dict(sel=sel, uvox=uvox, starts=starts,
                                    counts=counts, by_class=by_class))
            binned.append(streams)
    return binned


def _build_layout(binned):
    """Cross-core/stream padded class layout.

    Returns [(k, nwin, wins_per_chunk)], slot total L, window total V.
    Each class region is a whole number of chunks of wins_per_chunk
    windows (chunk columns = wins_per_chunk * k <= 512)."""
    classes = sorted({k for cs in binned for s in cs for k in s["by_class"]})
    layout = []
    for k in classes:
        wpc = CHUNK // k
        nw = max(len(s["by_class"].get(k, ())) for cs in binned for s in cs)
        nw = -(-nw // wpc) * wpc
        layout.append((k, nw, wpc))
    L = sum(k * nw for k, nw, _ in layout)
    V = sum(nw for _, nw, _ in layout)
    return layout, L, V


def _build_core_inputs(p_all, binned, layout, L, V):
    assert V + 1 <= 32768
    cores = []
    for core in range(NCORES):
        b, h = divmod(core, 2)
        p_f4 = np.zeros((12, L), dtype=np.float32)
        rank_map = np.full((NSTREAM, VOX_PER_STREAM), V, dtype=np.int32)
        for q in range(NSTREAM):
            s = binned[core][q]
            sel, uvox = s["sel"], s["uvox"]
            starts, counts = s["starts"], s["counts"]
            assert len(sel) > 0
            slot_pts = np.full(L, sel[0], dtype=np.int64)
            off_slot = 0
            off_win = 0
            for k, nw, _ in layout:
                for wi, ui in enumerate(s["by_class"].get(k, ())):
                    st, ct = starts[ui], counts[ui]
                    pts = sel[st:st + ct]
                    sl = off_slot + wi * k
                    slot_pts[sl:sl + k] = pts[0]
                    slot_pts[sl:sl + ct] = pts
                    rank_map[q, uvox[ui]] = off_win + wi
                off_slot += k * nw
                off_win += nw
            p_f4[3 * q:3 * q + 3, :] = p_all[b, slot_pts, :].T
        # inverse rank map for the sparse scatter: per (stream, window-tile)
        # column of row offsets into the [131072, 32] output (4*vloc + q),
        # pad windows get a huge offset (skipped via bounds_check)
        ntiles = -(-V // 128)
        voxoff = np.full((128, NSTREAM * ntiles), 1 << 20, dtype=np.int32)
        for q in range(NSTREAM):
            inv = np.full(V, 1 << 20, dtype=np.int64)
            occ = rank_map[q] < V
            vloc = np.nonzero(occ)[0]
            inv[rank_map[q][vloc]] = 4 * vloc + q
            for t in range(ntiles):
                w = inv[128 * t:128 * (t + 1)]
                voxoff[:len(w), q * ntiles + t] = w
        cores.append(dict(p_f4=p_f4, voxoff=voxoff))
    return cores


def _bd4(w):
    out = np.zeros((128, 128), dtype=np.float32)
    for q in range(4):
        out[32 * q:32 * q + 32, 32 * q:32 * q + 32] = w
    return out


def _bias_f4(bvec):
    return np.tile(np.asarray(bvec, np.float32), 4).reshape(128, 1)


def _build_weights(inp):
    W = {}
    W_pos = np.asarray(inp["W_pos"], np.float32)
    for half, sl in (("lo", slice(0, 32)), ("hi", slice(32, 64))):
        w = np.zeros((12, 128), dtype=np.float32)
        for q in range(4):
            w[3 * q:3 * q + 3, 32 * q:32 * q + 32] = W_pos[:, sl]
        W[f"wpos_{half}"] = w
    W["bpos_lo"] = _bias_f4(np.asarray(inp["b_pos"], np.float32)[:32])
    W["bpos_hi"] = _bias_f4(np.asarray(inp["b_pos"], np.float32)[32:])
    W0 = np.asarray(inp["W0"], np.float32)
    W1 = np.asarray(inp["W1"], np.float32)
    Ws = np.asarray(inp["Ws"], np.float32)
    for i in range(NB):
        W[f"w0a_{i}"] = _bd4(W0[i, :32])
        W[f"w0b_{i}"] = _bd4(W0[i, 32:])
        W[f"w0ab_{i}"] = _bd4(W0[i, :32] + W0[i, 32:])
        W[f"w1_{i}"] = _bd4(W1[i])
        W[f"wsa_{i}"] = _bd4(Ws[i, :32])
        W[f"wsb_{i}"] = _bd4(Ws[i, 32:])
        W[f"wsab_{i}"] = _bd4(Ws[i, :32] + Ws[i, 32:])
        W[f"b0_{i}"] = _bias_f4(inp["b0"][i])
        W[f"b1_{i}"] = _bias_f4(inp["b1"][i])
    W["wc"] = _bd4(np.asarray(inp["W_c"], np.float32))
    W["bc"] = _bias_f4(inp["b_c"])
    return W


WNAMES = (["wpos_lo", "wpos_hi", "wc"]
          + [f"{nm}_{i}" for i in range(NB)
             for nm in ("w0a", "w0b", "w0ab", "w1", "wsa", "wsb", "wsab")])
BNAMES = (["bpos_lo", "bpos_hi", "bc"]
          + [f"b0_{i}" for i in range(NB)] + [f"b1_{i}" for i in range(NB)])


def _emit_program(layout, L, V, MM_BF16):
    from concourse import bacc, bass, mybir, tile
    from concourse.masks import make_identity

    f32 = mybir.dt.float32
    mmdt = mybir.dt.bfloat16 if MM_BF16 else f32
    add = mybir.AluOpType.add
    relu = mybir.ActivationFunctionType.Relu

    nc = bacc.Bacc("TRN2", target_bir_lowering=False, debug=False,
                   num_devices=NCORES)

    d_p = nc.dram_tensor("p_f4", [12, L], mmdt, kind="ExternalInput")
    ntiles = -(-V // 128)
    d_voxoff = nc.dram_tensor("voxoff", [128, NSTREAM * ntiles],
                              mybir.dt.int32, kind="ExternalInput")
    d_w = {}
    for nm in WNAMES:
        shape = [12, 128] if nm.startswith("wpos") else [128, 128]
        d_w[nm] = nc.dram_tensor(nm, shape, mmdt, kind="ExternalInput")
    for nm in BNAMES:
        d_w[nm] = nc.dram_tensor(nm, [128, 1], f32, kind="ExternalInput")
    d_out = nc.dram_tensor("grid", [4 * VOX_PER_STREAM, 32], f32,
                           kind="ExternalOutput")

    # chunk bookkeeping
    chunk_info = []
    class_off = {}
    off_slot = 0
    off_win = 0
    for k, nw, wpc in layout:
        class_off[k] = (off_slot, off_win, nw)
        for c in range(nw // wpc):
            chunk_info.append(dict(
                k=k, slot0=off_slot + c * wpc * k, win0=off_win + c * wpc,
                nwin=wpc, cols=wpc * k))
        off_slot += nw * k
        off_win += nw

    with tile.TileContext(nc) as tc:
        with tc.tile_pool(name="persist", bufs=1) as pers, \
             tc.tile_pool(name="chunks", bufs=4) as chp, \
             tc.tile_pool(name="mpool", bufs=1) as mp, \
             tc.tile_pool(name="gridp", bufs=2) as grp, \
             tc.tile_pool(name="psum", bufs=2, space="PSUM") as psp:

            sb_w = {}
            for nm in WNAMES:
                shape = [12, 128] if nm.startswith("wpos") else [128, 128]
                t = pers.tile(shape, mmdt, tag=nm)
                nc.sync.dma_start(out=t[:], in_=d_w[nm][:])
                sb_w[nm] = t
            for nm in BNAMES:
                t = pers.tile([128, 1], f32, tag=nm)
                nc.sync.dma_start(out=t[:], in_=d_w[nm][:])
                sb_w[nm] = t
            voxoff_sb = pers.tile([128, NSTREAM * ntiles], mybir.dt.int32,
                                  tag="voxoff")
            nc.sync.dma_start(out=voxoff_sb[:], in_=d_voxoff[:])
            ident = pers.tile([128, 128], f32, tag="ident")
            make_identity(nc, ident[:])
            # zero-fill the output grid early; overlaps the whole MLP phase
            zt = pers.tile([128, 1024], f32, tag="zt")
            nc.vector.memset(zt[:], 0.0)
            zview = d_out[:].rearrange("(p a) c -> p a c", p=128)
            for j in range(32):
                nc.sync.dma_start(
                    out=zview[:, 32 * j:32 * (j + 1), :],
                    in_=zt[:].rearrange("p (a c) -> p a c", c=32))

            bigA = pers.tile([128, L], mmdt, tag="bigA")
            bigB = pers.tile([128, L], mmdt, tag="bigB")
            if MM_BF16:
                table_t = pers.tile([128, V + 1], f32, tag="table")
                table = table_t[:]
            else:
                # final phase: cur=bigA (net5), bigB (net4) is dead -> reuse
                table = bigB[:, 0:V + 1]

            def mm(ps, wname, rhs, start, stop):
                nc.tensor.matmul(out=ps, lhsT=sb_w[wname][:], rhs=rhs,
                                 start=start, stop=stop)

            # ---- fused pos MLP + block 0 (chunk-local X) ----
            for ci, info in enumerate(chunk_info):
                s0, cols = info["slot0"], info["cols"]
                p_ch = chp.tile([12, CHUNK], mmdt, tag="p_ch")
                nc.sync.dma_start(out=p_ch[:, :cols],
                                  in_=d_p[:, s0:s0 + cols])
                xlo = chp.tile([128, CHUNK], mmdt, tag="c0")
                xhi = chp.tile([128, CHUNK], mmdt, tag="c1")
                for wn, bn, X in (("wpos_lo", "bpos_lo", xlo),
                                  ("wpos_hi", "bpos_hi", xhi)):
                    ps = psp.tile([128, CHUNK], f32, space="PSUM", tag="psP")
                    mm(ps[:, :cols], wn, p_ch[:, :cols], True, True)
                    nc.vector.tensor_scalar_add(
                        out=X[:, :cols], in0=ps[:, :cols],
                        scalar1=sb_w[bn][:, 0:1])
                rlo = chp.tile([128, CHUNK], mmdt, tag="c2")
                rhi = chp.tile([128, CHUNK], mmdt, tag="c3")
                nc.scalar.activation(rlo[:, :cols], xlo[:, :cols], relu)
                nc.scalar.activation(rhi[:, :cols], xhi[:, :cols], relu)
                psA = psp.tile([128, CHUNK], f32, space="PSUM", tag="psA")
                mm(psA[:, :cols], "w0a_0", rlo[:, :cols], True, False)
                mm(psA[:, :cols], "w0b_0", rhi[:, :cols], False, True)
                ra = chp.tile([128, CHUNK], mmdt, tag="c4")
                nc.scalar.activation(ra[:, :cols], psA[:, :cols], relu,
                                     bias=sb_w["b0_0"][:, 0:1])
                psD = psp.tile([128, CHUNK], f32, space="PSUM", tag="psD")
                mm(psD[:, :cols], "w1_0", ra[:, :cols], True, False)
                mm(psD[:, :cols], "wsa_0", xlo[:, :cols], False, False)
                mm(psD[:, :cols], "wsb_0", xhi[:, :cols], False, True)
                nc.vector.tensor_scalar_add(
                    out=bigA[:, s0:s0 + cols], in0=psD[:, :cols],
                    scalar1=sb_w["b1_0"][:, 0:1])

            # ---- blocks 1..4 with pooling ----
            cur, nxt = bigA, bigB
            for i in range(1, NB):
                M, RM = {}, {}
                for k, nw, wpc in layout:
                    if k == 1:
                        continue
                    slot0, win0, nwk = class_off[k]
                    m = mp.tile([128, nwk], mmdt, tag=f"M_{k}")
                    nc.vector.tensor_reduce(
                        out=m[:],
                        in_=cur[:, slot0:slot0 + nwk * k]
                        .rearrange("p (w k) -> p w k", k=k),
                        axis=mybir.AxisListType.X, op=mybir.AluOpType.max)
                    rm = mp.tile([128, nwk], mmdt, tag=f"RM_{k}")
                    nc.scalar.activation(rm[:], m[:], relu)
                    M[k], RM[k] = m, rm
                for ci, info in enumerate(chunk_info):
                    k, s0, cols = info["k"], info["slot0"], info["cols"]
                    rn = chp.tile([128, CHUNK], mmdt, tag="c0")
                    nc.scalar.activation(rn[:, :cols], cur[:, s0:s0 + cols],
                                         relu)
                    psA = psp.tile([128, CHUNK], f32, space="PSUM", tag="psA")
                    psD = psp.tile([128, CHUNK], f32, space="PSUM", tag="psD")
                    if k == 1:
                        mm(psA[:, :cols], f"w0ab_{i}", rn[:, :cols],
                           True, True)
                    else:
                        wrel = info["win0"] - class_off[k][1]
                        mm(psA[:, :cols], f"w0a_{i}", rn[:, :cols],
                           True, False)
                        bc = RM[k][:, wrel:wrel + info["nwin"]] \
                            .unsqueeze(2).to_broadcast([128, info["nwin"], k])
                        mm(psA[:, :cols], f"w0b_{i}", bc, False, True)
                    ra = chp.tile([128, CHUNK], mmdt, tag="c4")
                    nc.scalar.activation(ra[:, :cols], psA[:, :cols], relu,
                                         bias=sb_w[f"b0_{i}"][:, 0:1])
                    mm(psD[:, :cols], f"w1_{i}", ra[:, :cols], True, False)
                    if k == 1:
                        mm(psD[:, :cols], f"wsab_{i}", cur[:, s0:s0 + cols],
                           False, True)
                    else:
                        wrel = info["win0"] - class_off[k][1]
                        mm(psD[:, :cols], f"wsa_{i}", cur[:, s0:s0 + cols],
                           False, False)
                        bc = M[k][:, wrel:wrel + info["nwin"]] \
                            .unsqueeze(2).to_broadcast([128, info["nwin"], k])
                        mm(psD[:, :cols], f"wsb_{i}", bc, False, True)
                    nc.vector.tensor_scalar_add(
                        out=nxt[:, s0:s0 + cols], in0=psD[:, :cols],
                        scalar1=sb_w[f"b1_{i}"][:, 0:1])
                cur, nxt = nxt, cur

            # ---- final: c = net@W_c + b_c, per-voxel max, relu -> table ----
            for ci, info in enumerate(chunk_info):
                k, s0, cols = info["k"], info["slot0"], info["cols"]
                w0, nwin = info["win0"], info["nwin"]
                ps = psp.tile([128, CHUNK], f32, space="PSUM", tag="psC")
                mm(ps[:, :cols], "wc", cur[:, s0:s0 + cols], True, True)
                if k == 1:
                    nc.scalar.activation(table[:, w0:w0 + nwin],
                                         ps[:, :cols], relu,
                                         bias=sb_w["bc"][:, 0:1])
                else:
                    mc = chp.tile([128, CHUNK], f32, tag="c0")
                    nc.vector.tensor_reduce(
                        out=mc[:, :nwin],
                        in_=ps[:, :cols].rearrange("p (w k) -> p w k", k=k),
                        axis=mybir.AxisListType.X, op=mybir.AluOpType.max)
                    nc.scalar.activation(table[:, w0:w0 + nwin],
                                         mc[:, :nwin], relu,
                                         bias=sb_w["bc"][:, 0:1])
            # ---- sparse scatter of the per-voxel table into the grid ----
            for t in range(ntiles):
                w = min(128, V - 128 * t)
                tp = psp.tile([128, 128], f32, space="PSUM", tag="psA")
                nc.tensor.transpose(
                    out=tp[:w, :], in_=table[:, 128 * t:128 * t + w],
                    identity=ident[:])
                tt = grp.tile([128, 128], f32, tag="tt")
                nc.vector.tensor_copy(out=tt[:w, :], in_=tp[:w, :])
                for q in range(NSTREAM):
                    nc.gpsimd.indirect_dma_start(
                        out=d_out[:],
                        out_offset=bass.IndirectOffsetOnAxis(
                            ap=voxoff_sb[:, q * ntiles + t:
                                         q * ntiles + t + 1], axis=0),
                        in_=tt[:, 32 * q:32 * q + 32],
                        in_offset=None,
                        bounds_check=4 * VOX_PER_STREAM - 1,
                        oob_is_err=False)

    nc.compile()
    return nc


_CACHE = {}


def _to_mm_dtype(arr, MM_BF16):
    if MM_BF16:
        import ml_dtypes
        return np.asarray(arr).astype(ml_dtypes.bfloat16)
    return np.asarray(arr, np.float32)


def kernel(**inputs):
    from concourse.bass_utils import run_bass_kernel_spmd

    p_all = np.asarray(inputs["p"], np.float32)
    binned = _bin_streams(p_all)
    layout, L, V = _build_layout(binned)
    cores = _build_core_inputs(p_all, binned, layout, L, V)
    W = _build_weights(inputs)

    MM_BF16 = _pick_bf16(L)
    key = (tuple(layout), L, V, MM_BF16)
    if key not in _CACHE:
        _CACHE[key] = _emit_program(layout, L, V, MM_BF16)
    nc = _CACHE[key]

    in_maps = []
    for core in range(NCORES):
        m = {"p_f4": _to_mm_dtype(cores[core]["p_f4"], MM_BF16),
             "voxoff": cores[core]["voxoff"]}
        for nm in WNAMES:
            m[nm] = _to_mm_dtype(W[nm], MM_BF16)
        for nm in BNAMES:
            m[nm] = W[nm].astype(np.float32)
        in_maps.append(m)

    res = run_bass_kernel_spmd(nc, in_maps, list(range(NCORES)))

    out = np.zeros((B, 32, R, R, R), dtype=np.float32)
    for core in range(NCORES):
        b, h = divmod(core, 2)
        g = res.results[core]["grid"]  # [131072, 32] = (vloc, stream, ch)
        g = g.reshape(32768, 4, 32).transpose(1, 2, 0)  # [q, ch, vloc]
        g = g.reshape(4, 32, 8, 64, 64).transpose(1, 0, 2, 3, 4)
        out[b, :, 32 * h:32 * h + 32] = g.reshape(32, 32, 64, 64)
    return out


# revision 15
# speedup vs baseline: 4.4321x; 1.0128x over previous
"""Trainium2 Bass kernel for nn_LocalPoolPointNet (gnn_message_passing).

Sharding strategy (hardcoded):
  - 8 NeuronCores = 4 batches x 2 z-halves of the 64^3 grid. Points are
    sharded to the core owning their voxel's z-half, so every segment_max
    is core-local (a voxel's points all live on exactly one core) and no
    collective is needed.
  - Within a core, 4 "streams" (8 z-slices each) are folded across the 128
    SBUF partitions: partition 32*q + ch holds channel ch of stream q.
    Matmuls use block-diagonal [128,128] stationary weights so all 4
    streams multiply in a single PE pass at full array width.
  - Points are binned by voxel into fixed-size windows (slot class sizes
    1,2,3,4,6,8,... padded by duplicating a point of the same voxel, which
    is max-neutral). segment_max becomes a windowed DVE tensor_reduce(max)
    and the gather-back is a stride-0 access-pattern broadcast consumed
    directly by the PE as the moving matmul operand (no data movement).
    Single-point voxels (the majority) skip pooling entirely: their pooled
    value equals the point value, folded in by pre-summing the net/pooled
    weight blocks.
  - The final dense [64^3, ch] grid is zero-filled by early (overlapped)
    DMAs; the compressed per-voxel table is PE-transposed to voxel-major
    and scattered sparsely with indirect DMA (pad windows skipped via
    bounds_check).
"""

import os
import sys

sys.path.insert(0, "/opt/trn_rl_repo")

import numpy as np

R = 64
B = 4
N = 100000
NB = 5
NCORES = 8
NSTREAM = 4
VOX_PER_STREAM = 64 * 64 * 8  # 32768
CHUNK = 512
CLASS_SIZES = (1, 2, 3, 4, 6, 8, 12, 16, 24, 32, 48, 64, 96, 128, 160, 192,
               224, 256, 320, 384, 448, 512)
GRID_CHUNK = 2048  # columns per ap_gather / output DMA chunk

# fp32 fits SBUF only for compact layouts; clustered data needs bf16
FP32_MAX_L = 13824


def _pick_bf16(L):
    ov = os.environ.get("KERNEL_MM_BF16")
    if ov is not None:
        return ov == "1"
    return L > FP32_MAX_L


def _coords2index_np(p):
    """Exact float32 replica of reference._coords2index."""
    p = np.asarray(p, dtype=np.float32)
    pn = np.clip(p + np.float32(1.0), np.float32(0.0), np.float32(2.0 - 0.0001))
    xi = ((pn / np.float32(2.0)) * np.float32(R)).astype(np.int32)
    return xi[..., 0] + R * (xi[..., 1] + R * xi[..., 2])


def _class_of(occ):
    for k in CLASS_SIZES:
        if occ <= k:
            return k
    raise ValueError(f"voxel occupancy {occ} too large")


def _bin_streams(p_all):
    """Host-side sharding: bin points by (core, stream, voxel)."""
    idx_all = _coords2index_np(p_all)  # [B, N]
    binned = []
    for b in range(B):
        idx_b = idx_all[b]
        z = idx_b >> 12
        for h in range(2):
            streams = []
            for q in range(NSTREAM):
                z0 = 32 * h + 8 * q
                sel = np.nonzero((z >= z0) & (z < z0 + 8))[0]
                vloc = idx_b[sel] - 4096 * z0
                order = np.argsort(vloc, kind="stable")
                sel, vloc = sel[order], vloc[order]
                uvox, starts, counts = np.unique(
                    vloc, return_index=True, return_counts=True)
                by_class = {}
                for ui in range(len(uvox)):
                    by_class.setdefault(_class_of(counts[ui]), []).append(ui)
                streams.append(dict(sel=sel, uvox=uvox, starts=starts,
                                    counts=counts, by_class=by_class))
            binned.append(streams)
    return binned


def _build_layout(binned):
    """Cross-core/stream padded class layout.

    Returns [(k, nwin, wins_per_chunk)], slot total L, window total V.
    Each class region is a whole number of chunks of wins_per_chunk
    windows (chunk columns = wins_per_chunk * k <= 512)."""
    classes = sorted({k for cs in binned for s in cs for k in s["by_class"]})
    layout = []
    for k in classes:
        wpc = CHUNK // k
        nw = max(len(s["by_class"].get(k, ())) for cs in binned for s in cs)
        nw = -(-nw // wpc) * wpc
        layout.append((k, nw, wpc))
    L = sum(k * nw for k, nw, _ in layout)
    V = sum(nw for _, nw, _ in layout)
    return layout, L, V


def _build_core_inputs(p_all, binned, layout, L, V):
    assert V + 1 <= 32768
    cores = []
    for core in range(NCORES):
        b, h = divmod(core, 2)
        p_f4 = np.zeros((12, L), dtype=np.float32)
        rank_map = np.full((NSTREAM, VOX_PER_STREAM), V, dtype=np.int32)
        for q in range(NSTREAM):
            s = binned[core][q]
            sel, uvox = s["sel"], s["uvox"]
            starts, counts = s["starts"], s["counts"]
            assert len(sel) > 0
            slot_pts = np.full(L, sel[0], dtype=np.int64)
            off_slot = 0
            off_win = 0
            for k, nw, _ in layout:
                for wi, ui in enumerate(s["by_class"].get(k, ())):
                    st, ct = starts[ui], counts[ui]
                    pts = sel[st:st + ct]
                    sl = off_slot + wi * k
                    slot_pts[sl:sl + k] = pts[0]
                    slot_pts[sl:sl + ct] = pts
                    rank_map[q, uvox[ui]] = off_win + wi
                off_slot += k * nw
                off_win += nw
            p_f4[3 * q:3 * q + 3, :] = p_all[b, slot_pts, :].T
        # inverse rank map for the sparse scatter: per (stream, window-tile)
        # column of row offsets into the [131072, 32] output (4*vloc + q),
        # pad windows get a huge offset (skipped via bounds_check)
        ntiles = -(-V // 128)
        voxoff = np.full((128, NSTREAM * ntiles), 1 << 20, dtype=np.int32)
        for q in range(NSTREAM):
            inv = np.full(V, 1 << 20, dtype=np.int64)
            occ = rank_map[q] < V
            vloc = np.nonzero(occ)[0]
            inv[rank_map[q][vloc]] = 4 * vloc + q
            for t in range(ntiles):
                w = inv[128 * t:128 * (t + 1)]
                voxoff[:len(w), q * ntiles + t] = w
        cores.append(dict(p_f4=p_f4, voxoff=voxoff))
    return cores


def _bd4(w):
    out = np.zeros((128, 128), dtype=np.float32)
    for q in range(4):
        out[32 * q:32 * q + 32, 32 * q:32 * q + 32] = w
    return out


def _bias_f4(bvec):
    return np.tile(np.asarray(bvec, np.float32), 4).reshape(128, 1)


def _build_weights(inp):
    W = {}
    W_pos = np.asarray(inp["W_pos"], np.float32)
    for half, sl in (("lo", slice(0, 32)), ("hi", slice(32, 64))):
        w = np.zeros((12, 128), dtype=np.float32)
        for q in range(4):
            w[3 * q:3 * q + 3, 32 * q:32 * q + 32] = W_pos[:, sl]
        W[f"wpos_{half}"] = w
    W["bpos_lo"] = _bias_f4(np.asarray(inp["b_pos"], np.float32)[:32])
    W["bpos_hi"] = _bias_f4(np.asarray(inp["b_pos"], np.float32)[32:])
    W0 = np.asarray(inp["W0"], np.float32)
    W1 = np.asarray(inp["W1"], np.float32)
    Ws = np.asarray(inp["Ws"], np.float32)
    for i in range(NB):
        W[f"w0a_{i}"] = _bd4(W0[i, :32])
        W[f"w0b_{i}"] = _bd4(W0[i, 32:])
        W[f"w0ab_{i}"] = _bd4(W0[i, :32] + W0[i, 32:])
        W[f"w1_{i}"] = _bd4(W1[i])
        W[f"wsa_{i}"] = _bd4(Ws[i, :32])
        W[f"wsb_{i}"] = _bd4(Ws[i, 32:])
        W[f"wsab_{i}"] = _bd4(Ws[i, :32] + Ws[i, 32:])
        W[f"b0_{i}"] = _bias_f4(inp["b0"][i])
        b1 = np.asarray(inp["b1"][i], np.float32)
        if i == 0:
            b1 = b1 + np.asarray(inp["b_pos"], np.float32) @ Ws[0]
        W[f"b1_{i}"] = _bias_f4(b1)
    W["wc"] = _bd4(np.asarray(inp["W_c"], np.float32))
    W["bc"] = _bias_f4(inp["b_c"])
    return W


WNAMES = (["wpos_lo", "wpos_hi", "wc"]
          + [f"{nm}_{i}" for i in range(NB)
             for nm in ("w0a", "w0b", "w0ab", "w1", "wsa", "wsb", "wsab")])
BNAMES = (["bpos_lo", "bpos_hi", "bc"]
          + [f"b0_{i}" for i in range(NB)] + [f"b1_{i}" for i in range(NB)])


def _emit_program(layout, L, V, MM_BF16):
    from concourse import bacc, bass, mybir, tile
    from concourse.masks import make_identity

    f32 = mybir.dt.float32
    mmdt = mybir.dt.bfloat16 if MM_BF16 else f32
    add = mybir.AluOpType.add
    relu = mybir.ActivationFunctionType.Relu

    nc = bacc.Bacc("TRN2", target_bir_lowering=False, debug=False,
                   num_devices=NCORES)

    d_p = nc.dram_tensor("p_f4", [12, L], mmdt, kind="ExternalInput")
    ntiles = -(-V // 128)
    d_voxoff = nc.dram_tensor("voxoff", [128, NSTREAM * ntiles],
                              mybir.dt.int32, kind="ExternalInput")
    d_w = {}
    for nm in WNAMES:
        shape = [12, 128] if nm.startswith("wpos") else [128, 128]
        d_w[nm] = nc.dram_tensor(nm, shape, mmdt, kind="ExternalInput")
    for nm in BNAMES:
        d_w[nm] = nc.dram_tensor(nm, [128, 1], f32, kind="ExternalInput")
    d_out = nc.dram_tensor("grid", [4 * VOX_PER_STREAM, 32], f32,
                           kind="ExternalOutput")

    # chunk bookkeeping
    chunk_info = []
    class_off = {}
    off_slot = 0
    off_win = 0
    for k, nw, wpc in layout:
        class_off[k] = (off_slot, off_win, nw)
        for c in range(nw // wpc):
            chunk_info.append(dict(
                k=k, slot0=off_slot + c * wpc * k, win0=off_win + c * wpc,
                nwin=wpc, cols=wpc * k))
        off_slot += nw * k
        off_win += nw

    with tile.TileContext(nc) as tc:
        with tc.tile_pool(name="persist", bufs=1) as pers, \
             tc.tile_pool(name="chunks", bufs=4) as chp, \
             tc.tile_pool(name="mpool", bufs=1) as mp, \
             tc.tile_pool(name="gridp", bufs=2) as grp, \
             tc.tile_pool(name="psum", bufs=3, space="PSUM") as psp, \
             tc.tile_pool(name="psum2", bufs=1, space="PSUM") as psp2:

            sb_w = {}
            for nm in WNAMES:
                shape = [12, 128] if nm.startswith("wpos") else [128, 128]
                t = pers.tile(shape, mmdt, tag=nm)
                nc.sync.dma_start(out=t[:], in_=d_w[nm][:])
                sb_w[nm] = t
            for nm in BNAMES:
                t = pers.tile([128, 1], f32, tag=nm)
                nc.sync.dma_start(out=t[:], in_=d_w[nm][:])
                sb_w[nm] = t
            voxoff_sb = pers.tile([128, NSTREAM * ntiles], mybir.dt.int32,
                                  tag="voxoff")
            nc.sync.dma_start(out=voxoff_sb[:], in_=d_voxoff[:])
            ident = pers.tile([128, 128], f32, tag="ident")
            make_identity(nc, ident[:])
            # zero-fill the output grid early; overlaps the whole MLP phase
            zt = pers.tile([128, 1024], f32, tag="zt")
            nc.vector.memset(zt[:], 0.0)
            zview = d_out[:].rearrange("(p a) c -> p a c", p=128)
            for j in range(32):
                nc.sync.dma_start(
                    out=zview[:, 32 * j:32 * (j + 1), :],
                    in_=zt[:].rearrange("p (a c) -> p a c", c=32))

            bigA = pers.tile([128, L], mmdt, tag="bigA")
            bigB = pers.tile([128, L], mmdt, tag="bigB")
            rn_full = pers.tile([128, L], mmdt, tag="rn_full")
            if MM_BF16:
                table_t = pers.tile([128, V + 1], f32, tag="table")
                table = table_t[:]
            else:
                # final phase: cur=bigA (net5), bigB (net4) is dead -> reuse
                table = bigB[:, 0:V + 1]

            def mm(ps, wname, rhs, start, stop):
                nc.tensor.matmul(out=ps, lhsT=sb_w[wname][:], rhs=rhs,
                                 start=start, stop=stop)

            # ---- fused pos MLP + block 0 (chunk-local X) ----
            for ci, info in enumerate(chunk_info):
                s0, cols = info["slot0"], info["cols"]
                p_ch = chp.tile([12, CHUNK], mmdt, tag="p_ch")
                nc.sync.dma_start(out=p_ch[:, :cols],
                                  in_=d_p[:, s0:s0 + cols])
                xlo = chp.tile([128, CHUNK], mmdt, tag="c0")
                xhi = chp.tile([128, CHUNK], mmdt, tag="c1")
                for wn, bn, X in (("wpos_lo", "bpos_lo", xlo),
                                  ("wpos_hi", "bpos_hi", xhi)):
                    ps = psp2.tile([128, CHUNK], f32, space="PSUM", tag="psP")
                    mm(ps[:, :cols], wn, p_ch[:, :cols], True, True)
                    nc.scalar.activation(X[:, :cols], ps[:, :cols],
                                         mybir.ActivationFunctionType.Copy)
                rlo = chp.tile([128, CHUNK], mmdt, tag="c2")
                rhi = chp.tile([128, CHUNK], mmdt, tag="c3")
                nc.scalar.activation(rlo[:, :cols], xlo[:, :cols], relu,
                                     bias=sb_w["bpos_lo"][:, 0:1])
                nc.scalar.activation(rhi[:, :cols], xhi[:, :cols], relu,
                                     bias=sb_w["bpos_hi"][:, 0:1])
                psA = psp.tile([128, CHUNK], f32, space="PSUM", tag="psA")
                mm(psA[:, :cols], "w0a_0", rlo[:, :cols], True, False)
                mm(psA[:, :cols], "w0b_0", rhi[:, :cols], False, True)
                ra = chp.tile([128, CHUNK], mmdt, tag="c4")
                nc.scalar.activation(ra[:, :cols], psA[:, :cols], relu,
                                     bias=sb_w["b0_0"][:, 0:1])
                psD = psp.tile([128, CHUNK], f32, space="PSUM", tag="psD")
                mm(psD[:, :cols], "w1_0", ra[:, :cols], True, False)
                mm(psD[:, :cols], "wsa_0", xlo[:, :cols], False, False)
                mm(psD[:, :cols], "wsb_0", xhi[:, :cols], False, True)
                nc.vector.tensor_scalar_add(
                    out=bigA[:, s0:s0 + cols], in0=psD[:, :cols],
                    scalar1=sb_w["b1_0"][:, 0:1])

            # ---- blocks 1..4 with pooling ----
            cur, nxt = bigA, bigB
            for i in range(1, NB):
                RNW = 2048
                for r0 in range(0, L, RNW):
                    r1 = min(L, r0 + RNW)
                    nc.scalar.activation(rn_full[:, r0:r1], cur[:, r0:r1],
                                         relu)
                M, RM = {}, {}
                for k, nw, wpc in layout:
                    if k == 1:
                        continue
                    slot0, win0, nwk = class_off[k]
                    m = mp.tile([128, nwk], mmdt, tag=f"M_{k}")
                    nc.vector.tensor_reduce(
                        out=m[:],
                        in_=cur[:, slot0:slot0 + nwk * k]
                        .rearrange("p (w k) -> p w k", k=k),
                        axis=mybir.AxisListType.X, op=mybir.AluOpType.max)
                    rm = mp.tile([128, nwk], mmdt, tag=f"RM_{k}")
                    nc.scalar.activation(rm[:], m[:], relu)
                    M[k], RM[k] = m, rm
                for ci, info in enumerate(chunk_info):
                    k, s0, cols = info["k"], info["slot0"], info["cols"]
                    rn = rn_full[:, s0:s0 + cols]
                    psA = psp.tile([128, CHUNK], f32, space="PSUM", tag="psA")
                    psD = psp.tile([128, CHUNK], f32, space="PSUM", tag="psD")
                    if k == 1:
                        mm(psA[:, :cols], f"w0ab_{i}", rn,
                           True, True)
                    else:
                        wrel = info["win0"] - class_off[k][1]
                        mm(psA[:, :cols], f"w0a_{i}", rn,
                           True, False)
                        bc = RM[k][:, wrel:wrel + info["nwin"]] \
                            .unsqueeze(2).to_broadcast([128, info["nwin"], k])
                        mm(psA[:, :cols], f"w0b_{i}", bc, False, True)
                    ra = chp.tile([128, CHUNK], mmdt, tag="c4")
                    nc.scalar.activation(ra[:, :cols], psA[:, :cols], relu,
                                         bias=sb_w[f"b0_{i}"][:, 0:1])
                    mm(psD[:, :cols], f"w1_{i}", ra[:, :cols], True, False)
                    if k == 1:
                        mm(psD[:, :cols], f"wsab_{i}", cur[:, s0:s0 + cols],
                           False, True)
                    else:
                        wrel = info["win0"] - class_off[k][1]
                        mm(psD[:, :cols], f"wsa_{i}", cur[:, s0:s0 + cols],
                           False, False)
                        bc = M[k][:, wrel:wrel + info["nwin"]] \
                            .unsqueeze(2).to_broadcast([128, info["nwin"], k])
                        mm(psD[:, :cols], f"wsb_{i}", bc, False, True)
                    nc.vector.tensor_scalar_add(
                        out=nxt[:, s0:s0 + cols], in0=psD[:, :cols],
                        scalar1=sb_w[f"b1_{i}"][:, 0:1])
                cur, nxt = nxt, cur

            # ---- final: c = net@W_c + b_c, per-voxel max, relu -> table ----
            for ci, info in enumerate(chunk_info):
                k, s0, cols = info["k"], info["slot0"], info["cols"]
                w0, nwin = info["win0"], info["nwin"]
                ps = psp2.tile([128, CHUNK], f32, space="PSUM", tag="psC")
                mm(ps[:, :cols], "wc", cur[:, s0:s0 + cols], True, True)
                if k == 1:
                    nc.scalar.activation(table[:, w0:w0 + nwin],
                                         ps[:, :cols], relu,
                                         bias=sb_w["bc"][:, 0:1])
                else:
                    mc = chp.tile([128, CHUNK], f32, tag="c0")
                    nc.vector.tensor_reduce(
                        out=mc[:, :nwin],
                        in_=ps[:, :cols].rearrange("p (w k) -> p w k", k=k),
                        axis=mybir.AxisListType.X, op=mybir.AluOpType.max)
                    nc.scalar.activation(table[:, w0:w0 + nwin],
                                         mc[:, :nwin], relu,
                                         bias=sb_w["bc"][:, 0:1])
            # ---- sparse scatter of the per-voxel table into the grid ----
            for t in range(ntiles):
                w = min(128, V - 128 * t)
                tp = psp2.tile([128, 128], f32, space="PSUM", tag="psP")
                nc.tensor.transpose(
                    out=tp[:w, :], in_=table[:, 128 * t:128 * t + w],
                    identity=ident[:])
                tt = grp.tile([128, 128], f32, tag="tt")
                nc.vector.tensor_copy(out=tt[:w, :], in_=tp[:w, :])
                for q in range(NSTREAM):
                    nc.gpsimd.indirect_dma_start(
                        out=d_out[:],
                        out_offset=bass.IndirectOffsetOnAxis(
                            ap=voxoff_sb[:, q * ntiles + t:
                                         q * ntiles + t + 1], axis=0),
                        in_=tt[:, 32 * q:32 * q + 32],
                        in_offset=None,
                        bounds_check=4 * VOX_PER_STREAM - 1,
                        oob_is_err=False)

    nc.compile()
    return nc


_CACHE = {}


def _to_mm_dtype(arr, MM_BF16):
    if MM_BF16:
        import ml_dtypes
        return np.asarray(arr).astype(ml_dtypes.bfloat16)
    return np.asarray(arr, np.float32)


def kernel(**inputs):
    from concourse.bass_utils import run_bass_kernel_spmd

    p_all = np.asarray(inputs["p"], np.float32)
    binned = _bin_streams(p_all)
    layout, L, V = _build_layout(binned)
    cores = _build_core_inputs(p_all, binned, layout, L, V)
    W = _build_weights(inputs)

    MM_BF16 = _pick_bf16(L)
    key = (tuple(layout), L, V, MM_BF16)
    if key not in _CACHE:
        _CACHE[key] = _emit_program(layout, L, V, MM_BF16)
    nc = _CACHE[key]

    in_maps = []
    for core in range(NCORES):
        m = {"p_f4": _to_mm_dtype(cores[core]["p_f4"], MM_BF16),
             "voxoff": cores[core]["voxoff"]}
        for nm in WNAMES:
            m[nm] = _to_mm_dtype(W[nm], MM_BF16)
        for nm in BNAMES:
            m[nm] = W[nm].astype(np.float32)
        in_maps.append(m)

    res = run_bass_kernel_spmd(nc, in_maps, list(range(NCORES)))

    out = np.zeros((B, 32, R, R, R), dtype=np.float32)
    for core in range(NCORES):
        b, h = divmod(core, 2)
        g = res.results[core]["grid"]  # [131072, 32] = (vloc, stream, ch)
        g = g.reshape(32768, 4, 32).transpose(1, 2, 0)  # [q, ch, vloc]
        g = g.reshape(4, 32, 8, 64, 64).transpose(1, 0, 2, 3, 4)
        out[b, :, 32 * h:32 * h + 32] = g.reshape(32, 32, 64, 64)
    return out


# revision 16
# speedup vs baseline: 4.5300x; 1.0221x over previous
"""Trainium2 Bass kernel for nn_LocalPoolPointNet (gnn_message_passing).

Sharding strategy (hardcoded):
  - 8 NeuronCores = 4 batches x 2 z-halves of the 64^3 grid. Points are
    sharded to the core owning their voxel's z-half, so every segment_max
    is core-local (a voxel's points all live on exactly one core) and no
    collective is needed.
  - Within a core, 4 "streams" (8 z-slices each) are folded across the 128
    SBUF partitions: partition 32*q + ch holds channel ch of stream q.
    Matmuls use block-diagonal [128,128] stationary weights so all 4
    streams multiply in a single PE pass at full array width.
  - Points are binned by voxel into fixed-size windows (slot class sizes
    1,2,3,4,6,8,... padded by duplicating a point of the same voxel, which
    is max-neutral). segment_max becomes a windowed DVE tensor_reduce(max)
    and the gather-back is a stride-0 access-pattern broadcast consumed
    directly by the PE as the moving matmul operand (no data movement).
    Single-point voxels (the majority) skip pooling entirely: their pooled
    value equals the point value, folded in by pre-summing the net/pooled
    weight blocks.
  - The final dense [64^3, ch] grid is zero-filled by early (overlapped)
    DMAs; the compressed per-voxel table is PE-transposed to voxel-major
    and scattered sparsely with indirect DMA (pad windows skipped via
    bounds_check).
"""

import os
import sys

sys.path.insert(0, "/opt/trn_rl_repo")

import numpy as np

R = 64
B = 4
N = 100000
NB = 5
NCORES = 8
NSTREAM = 4
VOX_PER_STREAM = 64 * 64 * 8  # 32768
CHUNK = 512
CLASS_SIZES = (1, 2, 3, 4, 6, 8, 12, 16, 24, 32, 48, 64, 96, 128, 160, 192,
               224, 256, 320, 384, 448, 512)
GRID_CHUNK = 2048  # columns per ap_gather / output DMA chunk

# fp32 fits SBUF only for compact layouts; clustered data needs bf16
FP32_MAX_L = 13824


def _pick_bf16(L):
    ov = os.environ.get("KERNEL_MM_BF16")
    if ov is not None:
        return ov == "1"
    return L > FP32_MAX_L


def _coords2index_np(p):
    """Exact float32 replica of reference._coords2index."""
    p = np.asarray(p, dtype=np.float32)
    pn = np.clip(p + np.float32(1.0), np.float32(0.0), np.float32(2.0 - 0.0001))
    xi = ((pn / np.float32(2.0)) * np.float32(R)).astype(np.int32)
    return xi[..., 0] + R * (xi[..., 1] + R * xi[..., 2])


def _class_of(occ):
    for k in CLASS_SIZES:
        if occ <= k:
            return k
    raise ValueError(f"voxel occupancy {occ} too large")


def _bin_streams(p_all):
    """Host-side sharding: bin points by (core, stream, voxel)."""
    idx_all = _coords2index_np(p_all)  # [B, N]
    binned = []
    for b in range(B):
        idx_b = idx_all[b]
        z = idx_b >> 12
        for h in range(2):
            streams = []
            for q in range(NSTREAM):
                z0 = 32 * h + 8 * q
                sel = np.nonzero((z >= z0) & (z < z0 + 8))[0]
                vloc = idx_b[sel] - 4096 * z0
                order = np.argsort(vloc, kind="stable")
                sel, vloc = sel[order], vloc[order]
                uvox, starts, counts = np.unique(
                    vloc, return_index=True, return_counts=True)
                by_class = {}
                for ui in range(len(uvox)):
                    by_class.setdefault(_class_of(counts[ui]), []).append(ui)
                streams.append(dict(sel=sel, uvox=uvox, starts=starts,
                                    counts=counts, by_class=by_class))
            binned.append(streams)
    return binned


def _build_layout(binned):
    """Cross-core/stream padded class layout.

    Returns [(k, nwin, wins_per_chunk)], slot total L, window total V.
    Each class region is a whole number of chunks of wins_per_chunk
    windows (chunk columns = wins_per_chunk * k <= 512)."""
    classes = sorted({k for cs in binned for s in cs for k in s["by_class"]})
    layout = []
    for k in classes:
        wpc = CHUNK // k
        nw = max(len(s["by_class"].get(k, ())) for cs in binned for s in cs)
        nw = -(-nw // wpc) * wpc
        layout.append((k, nw, wpc))
    L = sum(k * nw for k, nw, _ in layout)
    V = sum(nw for _, nw, _ in layout)
    return layout, L, V


def _build_core_inputs(p_all, binned, layout, L, V):
    assert V + 1 <= 32768
    cores = []
    for core in range(NCORES):
        b, h = divmod(core, 2)
        p_f4 = np.zeros((12, L), dtype=np.float32)
        rank_map = np.full((NSTREAM, VOX_PER_STREAM), V, dtype=np.int32)
        for q in range(NSTREAM):
            s = binned[core][q]
            sel, uvox = s["sel"], s["uvox"]
            starts, counts = s["starts"], s["counts"]
            assert len(sel) > 0
            slot_pts = np.full(L, sel[0], dtype=np.int64)
            off_slot = 0
            off_win = 0
            for k, nw, _ in layout:
                for wi, ui in enumerate(s["by_class"].get(k, ())):
                    st, ct = starts[ui], counts[ui]
                    pts = sel[st:st + ct]
                    sl = off_slot + wi * k
                    slot_pts[sl:sl + k] = pts[0]
                    slot_pts[sl:sl + ct] = pts
                    rank_map[q, uvox[ui]] = off_win + wi
                off_slot += k * nw
                off_win += nw
            p_f4[3 * q:3 * q + 3, :] = p_all[b, slot_pts, :].T
        # inverse rank map for the sparse scatter: per (stream, window-tile)
        # column of row offsets into the [131072, 32] output (4*vloc + q),
        # pad windows get a huge offset (skipped via bounds_check)
        ntiles = -(-V // 128)
        voxoff = np.full((128, NSTREAM * ntiles), 1 << 20, dtype=np.int32)
        for q in range(NSTREAM):
            inv = np.full(V, 1 << 20, dtype=np.int64)
            occ = rank_map[q] < V
            vloc = np.nonzero(occ)[0]
            inv[rank_map[q][vloc]] = 4 * vloc + q
            for t in range(ntiles):
                w = inv[128 * t:128 * (t + 1)]
                voxoff[:len(w), q * ntiles + t] = w
        cores.append(dict(p_f4=p_f4, voxoff=voxoff))
    return cores


def _bd4(w):
    out = np.zeros((128, 128), dtype=np.float32)
    for q in range(4):
        out[32 * q:32 * q + 32, 32 * q:32 * q + 32] = w
    return out


def _bias_f4(bvec):
    return np.tile(np.asarray(bvec, np.float32), 4).reshape(128, 1)


def _build_weights(inp):
    W = {}
    W_pos = np.asarray(inp["W_pos"], np.float32)
    for half, sl in (("lo", slice(0, 32)), ("hi", slice(32, 64))):
        w = np.zeros((12, 128), dtype=np.float32)
        for q in range(4):
            w[3 * q:3 * q + 3, 32 * q:32 * q + 32] = W_pos[:, sl]
        W[f"wpos_{half}"] = w
    W["bpos_lo"] = _bias_f4(np.asarray(inp["b_pos"], np.float32)[:32])
    W["bpos_hi"] = _bias_f4(np.asarray(inp["b_pos"], np.float32)[32:])
    W0 = np.asarray(inp["W0"], np.float32)
    W1 = np.asarray(inp["W1"], np.float32)
    Ws = np.asarray(inp["Ws"], np.float32)
    for i in range(NB):
        W[f"w0a_{i}"] = _bd4(W0[i, :32])
        W[f"w0b_{i}"] = _bd4(W0[i, 32:])
        W[f"w0ab_{i}"] = _bd4(W0[i, :32] + W0[i, 32:])
        W[f"w1_{i}"] = _bd4(W1[i])
        W[f"wsa_{i}"] = _bd4(Ws[i, :32])
        W[f"wsb_{i}"] = _bd4(Ws[i, 32:])
        W[f"wsab_{i}"] = _bd4(Ws[i, :32] + Ws[i, 32:])
        W[f"b0_{i}"] = _bias_f4(inp["b0"][i])
        b1 = np.asarray(inp["b1"][i], np.float32)
        if i == 0:
            b1 = b1 + np.asarray(inp["b_pos"], np.float32) @ Ws[0]
        W[f"b1_{i}"] = _bias_f4(b1)
    W["wc"] = _bd4(np.asarray(inp["W_c"], np.float32))
    W["bc"] = _bias_f4(inp["b_c"])
    return W


WNAMES = (["wpos_lo", "wpos_hi", "wc"]
          + [f"{nm}_{i}" for i in range(NB)
             for nm in ("w0a", "w0b", "w0ab", "w1", "wsa", "wsb", "wsab")])
BNAMES = (["bpos_lo", "bpos_hi", "bc"]
          + [f"b0_{i}" for i in range(NB)] + [f"b1_{i}" for i in range(NB)])


def _emit_program(layout, L, V, MM_BF16):
    from concourse import bacc, bass, mybir, tile
    from concourse.masks import make_identity

    f32 = mybir.dt.float32
    mmdt = mybir.dt.bfloat16 if MM_BF16 else f32
    add = mybir.AluOpType.add
    relu = mybir.ActivationFunctionType.Relu

    nc = bacc.Bacc("TRN2", target_bir_lowering=False, debug=False,
                   num_devices=NCORES)

    d_p = nc.dram_tensor("p_f4", [12, L], mmdt, kind="ExternalInput")
    ntiles = -(-V // 128)
    d_voxoff = nc.dram_tensor("voxoff", [128, NSTREAM * ntiles],
                              mybir.dt.int32, kind="ExternalInput")
    d_w = {}
    for nm in WNAMES:
        shape = [12, 128] if nm.startswith("wpos") else [128, 128]
        d_w[nm] = nc.dram_tensor(nm, shape, mmdt, kind="ExternalInput")
    for nm in BNAMES:
        d_w[nm] = nc.dram_tensor(nm, [128, 1], f32, kind="ExternalInput")
    d_out = nc.dram_tensor("grid", [4 * VOX_PER_STREAM, 32], f32,
                           kind="ExternalOutput")

    # chunk bookkeeping
    chunk_info = []
    class_off = {}
    off_slot = 0
    off_win = 0
    for k, nw, wpc in layout:
        class_off[k] = (off_slot, off_win, nw)
        for c in range(nw // wpc):
            chunk_info.append(dict(
                k=k, slot0=off_slot + c * wpc * k, win0=off_win + c * wpc,
                nwin=wpc, cols=wpc * k))
        off_slot += nw * k
        off_win += nw

    with tile.TileContext(nc) as tc:
        with tc.tile_pool(name="persist", bufs=1) as pers, \
             tc.tile_pool(name="chunks", bufs=6) as chp, \
             tc.tile_pool(name="mpool", bufs=1) as mp, \
             tc.tile_pool(name="gridp", bufs=2) as grp, \
             tc.tile_pool(name="psum", bufs=3, space="PSUM") as psp, \
             tc.tile_pool(name="psum2", bufs=1, space="PSUM") as psp2:

            sb_w = {}
            for nm in WNAMES:
                shape = [12, 128] if nm.startswith("wpos") else [128, 128]
                t = pers.tile(shape, mmdt, tag=nm)
                nc.sync.dma_start(out=t[:], in_=d_w[nm][:])
                sb_w[nm] = t
            for nm in BNAMES:
                t = pers.tile([128, 1], f32, tag=nm)
                nc.sync.dma_start(out=t[:], in_=d_w[nm][:])
                sb_w[nm] = t
            voxoff_sb = pers.tile([128, NSTREAM * ntiles], mybir.dt.int32,
                                  tag="voxoff")
            nc.sync.dma_start(out=voxoff_sb[:], in_=d_voxoff[:])
            ident = pers.tile([128, 128], f32, tag="ident")
            make_identity(nc, ident[:])
            # zero-fill the output grid early; overlaps the whole MLP phase
            zt = pers.tile([128, 1024], f32, tag="zt")
            nc.vector.memset(zt[:], 0.0)
            zview = d_out[:].rearrange("(p a) c -> p a c", p=128)
            for j in range(32):
                nc.gpsimd.dma_start(
                    out=zview[:, 32 * j:32 * (j + 1), :],
                    in_=zt[:].rearrange("p (a c) -> p a c", c=32))

            bigA = pers.tile([128, L], mmdt, tag="bigA")
            bigB = pers.tile([128, L], mmdt, tag="bigB")
            rn_full = pers.tile([128, L], mmdt, tag="rn_full")
            if MM_BF16:
                table_t = pers.tile([128, V + 1], f32, tag="table")
                table = table_t[:]
            else:
                # final phase: cur=bigA (net5), bigB (net4) is dead -> reuse
                table = bigB[:, 0:V + 1]

            def mm(ps, wname, rhs, start, stop):
                nc.tensor.matmul(out=ps, lhsT=sb_w[wname][:], rhs=rhs,
                                 start=start, stop=stop)

            # ---- fused pos MLP + block 0 (chunk-local X) ----
            for ci, info in enumerate(chunk_info):
                s0, cols = info["slot0"], info["cols"]
                p_ch = chp.tile([12, CHUNK], mmdt, tag="p_ch")
                nc.sync.dma_start(out=p_ch[:, :cols],
                                  in_=d_p[:, s0:s0 + cols])
                xlo = chp.tile([128, CHUNK], mmdt, tag="c0")
                xhi = chp.tile([128, CHUNK], mmdt, tag="c1")
                for wn, bn, X in (("wpos_lo", "bpos_lo", xlo),
                                  ("wpos_hi", "bpos_hi", xhi)):
                    ps = psp2.tile([128, CHUNK], f32, space="PSUM", tag="psP")
                    mm(ps[:, :cols], wn, p_ch[:, :cols], True, True)
                    nc.scalar.activation(X[:, :cols], ps[:, :cols],
                                         mybir.ActivationFunctionType.Copy)
                rlo = chp.tile([128, CHUNK], mmdt, tag="c2")
                rhi = chp.tile([128, CHUNK], mmdt, tag="c3")
                nc.scalar.activation(rlo[:, :cols], xlo[:, :cols], relu,
                                     bias=sb_w["bpos_lo"][:, 0:1])
                nc.scalar.activation(rhi[:, :cols], xhi[:, :cols], relu,
                                     bias=sb_w["bpos_hi"][:, 0:1])
                psA = psp.tile([128, CHUNK], f32, space="PSUM", tag="psA")
                mm(psA[:, :cols], "w0a_0", rlo[:, :cols], True, False)
                mm(psA[:, :cols], "w0b_0", rhi[:, :cols], False, True)
                ra = chp.tile([128, CHUNK], mmdt, tag="c4")
                nc.scalar.activation(ra[:, :cols], psA[:, :cols], relu,
                                     bias=sb_w["b0_0"][:, 0:1])
                psD = psp.tile([128, CHUNK], f32, space="PSUM", tag="psD")
                mm(psD[:, :cols], "w1_0", ra[:, :cols], True, False)
                mm(psD[:, :cols], "wsa_0", xlo[:, :cols], False, False)
                mm(psD[:, :cols], "wsb_0", xhi[:, :cols], False, True)
                nc.vector.tensor_scalar_add(
                    out=bigA[:, s0:s0 + cols], in0=psD[:, :cols],
                    scalar1=sb_w["b1_0"][:, 0:1])

            # ---- blocks 1..4 with pooling ----
            cur, nxt = bigA, bigB
            for i in range(1, NB):
                RNW = 2048
                for r0 in range(0, L, RNW):
                    r1 = min(L, r0 + RNW)
                    nc.scalar.activation(rn_full[:, r0:r1], cur[:, r0:r1],
                                         relu)
                M, RM = {}, {}
                for k, nw, wpc in layout:
                    if k == 1:
                        continue
                    slot0, win0, nwk = class_off[k]
                    m = mp.tile([128, nwk], mmdt, tag=f"M_{k}")
                    nc.vector.tensor_reduce(
                        out=m[:],
                        in_=cur[:, slot0:slot0 + nwk * k]
                        .rearrange("p (w k) -> p w k", k=k),
                        axis=mybir.AxisListType.X, op=mybir.AluOpType.max)
                    rm = mp.tile([128, nwk], mmdt, tag=f"RM_{k}")
                    nc.scalar.activation(rm[:], m[:], relu)
                    M[k], RM[k] = m, rm
                for ci, info in enumerate(chunk_info):
                    k, s0, cols = info["k"], info["slot0"], info["cols"]
                    rn = rn_full[:, s0:s0 + cols]
                    psA = psp.tile([128, CHUNK], f32, space="PSUM", tag="psA")
                    psD = psp.tile([128, CHUNK], f32, space="PSUM", tag="psD")
                    if k == 1:
                        mm(psA[:, :cols], f"w0ab_{i}", rn,
                           True, True)
                    else:
                        wrel = info["win0"] - class_off[k][1]
                        mm(psA[:, :cols], f"w0a_{i}", rn,
                           True, False)
                        bc = RM[k][:, wrel:wrel + info["nwin"]] \
                            .unsqueeze(2).to_broadcast([128, info["nwin"], k])
                        mm(psA[:, :cols], f"w0b_{i}", bc, False, True)
                    ra = chp.tile([128, CHUNK], mmdt, tag="c4")
                    nc.scalar.activation(ra[:, :cols], psA[:, :cols], relu,
                                         bias=sb_w[f"b0_{i}"][:, 0:1])
                    mm(psD[:, :cols], f"w1_{i}", ra[:, :cols], True, False)
                    if k == 1:
                        mm(psD[:, :cols], f"wsab_{i}", cur[:, s0:s0 + cols],
                           False, True)
                    else:
                        wrel = info["win0"] - class_off[k][1]
                        mm(psD[:, :cols], f"wsa_{i}", cur[:, s0:s0 + cols],
                           False, False)
                        bc = M[k][:, wrel:wrel + info["nwin"]] \
                            .unsqueeze(2).to_broadcast([128, info["nwin"], k])
                        mm(psD[:, :cols], f"wsb_{i}", bc, False, True)
                    nc.vector.tensor_scalar_add(
                        out=nxt[:, s0:s0 + cols], in0=psD[:, :cols],
                        scalar1=sb_w[f"b1_{i}"][:, 0:1])
                cur, nxt = nxt, cur

            # ---- final: c = net@W_c + b_c, per-voxel max, relu -> table ----
            for ci, info in enumerate(chunk_info):
                k, s0, cols = info["k"], info["slot0"], info["cols"]
                w0, nwin = info["win0"], info["nwin"]
                ps = psp2.tile([128, CHUNK], f32, space="PSUM", tag="psC")
                mm(ps[:, :cols], "wc", cur[:, s0:s0 + cols], True, True)
                if k == 1:
                    nc.scalar.activation(table[:, w0:w0 + nwin],
                                         ps[:, :cols], relu,
                                         bias=sb_w["bc"][:, 0:1])
                else:
                    mc = chp.tile([128, CHUNK], f32, tag="c0")
                    nc.vector.tensor_reduce(
                        out=mc[:, :nwin],
                        in_=ps[:, :cols].rearrange("p (w k) -> p w k", k=k),
                        axis=mybir.AxisListType.X, op=mybir.AluOpType.max)
                    nc.scalar.activation(table[:, w0:w0 + nwin],
                                         mc[:, :nwin], relu,
                                         bias=sb_w["bc"][:, 0:1])
            # ---- sparse scatter of the per-voxel table into the grid ----
            for t in range(ntiles):
                w = min(128, V - 128 * t)
                tp = psp2.tile([128, 128], f32, space="PSUM", tag="psP")
                nc.tensor.transpose(
                    out=tp[:w, :], in_=table[:, 128 * t:128 * t + w],
                    identity=ident[:])
                tt = grp.tile([128, 128], f32, tag="tt")
                nc.vector.tensor_copy(out=tt[:w, :], in_=tp[:w, :])
                for q in range(NSTREAM):
                    nc.gpsimd.indirect_dma_start(
                        out=d_out[:],
                        out_offset=bass.IndirectOffsetOnAxis(
                            ap=voxoff_sb[:, q * ntiles + t:
                                         q * ntiles + t + 1], axis=0),
                        in_=tt[:, 32 * q:32 * q + 32],
                        in_offset=None,
                        bounds_check=4 * VOX_PER_STREAM - 1,
                        oob_is_err=False)

    nc.compile()
    return nc


_CACHE = {}


def _to_mm_dtype(arr, MM_BF16):
    if MM_BF16:
        import ml_dtypes
        return np.asarray(arr).astype(ml_dtypes.bfloat16)
    return np.asarray(arr, np.float32)


def kernel(**inputs):
    from concourse.bass_utils import run_bass_kernel_spmd

    p_all = np.asarray(inputs["p"], np.float32)
    binned = _bin_streams(p_all)
    layout, L, V = _build_layout(binned)
    cores = _build_core_inputs(p_all, binned, layout, L, V)
    W = _build_weights(inputs)

    MM_BF16 = _pick_bf16(L)
    key = (tuple(layout), L, V, MM_BF16)
    if key not in _CACHE:
        _CACHE[key] = _emit_program(layout, L, V, MM_BF16)
    nc = _CACHE[key]

    in_maps = []
    for core in range(NCORES):
        m = {"p_f4": _to_mm_dtype(cores[core]["p_f4"], MM_BF16),
             "voxoff": cores[core]["voxoff"]}
        for nm in WNAMES:
            m[nm] = _to_mm_dtype(W[nm], MM_BF16)
        for nm in BNAMES:
            m[nm] = W[nm].astype(np.float32)
        in_maps.append(m)

    res = run_bass_kernel_spmd(nc, in_maps, list(range(NCORES)))

    out = np.zeros((B, 32, R, R, R), dtype=np.float32)
    for core in range(NCORES):
        b, h = divmod(core, 2)
        g = res.results[core]["grid"]  # [131072, 32] = (vloc, stream, ch)
        g = g.reshape(32768, 4, 32).transpose(1, 2, 0)  # [q, ch, vloc]
        g = g.reshape(4, 32, 8, 64, 64).transpose(1, 0, 2, 3, 4)
        out[b, :, 32 * h:32 * h + 32] = g.reshape(32, 32, 64, 64)
    return out
